# revision 1
# baseline (speedup 1.0000x reference)
"""Trainium2 Bass kernel for nn_Attention_22539988369511.

Dense transformer attention block (B=4, N=2048, C=1024, H=16, hd=64),
sharded over 8 NeuronCores with tensor parallelism over heads (2 heads
per core), AllToAll to re-shard from heads to tokens before the output
projection, host concatenation of per-core token slices.

Math notes (validated against the jax reference in a numpy mock):
 - x is fed pre-transposed as x^T [C, B*N] so every matmul contracts on
   the partition dim with no on-chip transposes.
 - q/k_norm_w are folded into the RoPE cos/sin tables on the host
   (legal since per-token RMS scaling commutes with rotation).
 - RMS factors r = sqrt(1/(sumsq + 64*eps)) omit the x8; the softmax
   scale folds it: q carries r_q (K=1 ones-outer-product broadcast
   matmul + multiply), while r_k rides the exp activation's
   per-partition scale AP after a K=1 transpose matmul (rhs=8.0 also
   folds the missing 8*8/8 scale).
 - Softmax skips max-subtraction: rms-normed scores are bounded, exp
   stays in fp32 range.
 - P@V appends a ones column to V so the softmax denominator falls out
   of the same matmul (M=65).
 - Pipeline is interleaved per batch: {qkv chunk, norm chunk} x2 then
   attention for that batch, so PE/ACT/DVE overlap across phases.
"""
import os
import sys

import numpy as np
import ml_dtypes

for _p in ("/opt/trn_rl_repo", "/root/.axon_site/_ro/trn_rl_repo"):
    if os.path.isdir(_p) and _p not in sys.path:
        sys.path.append(_p)

import concourse.bass as bass
import concourse.mybir as mybir
from concourse import bacc, tile
from concourse.bass_utils import run_bass_kernel_spmd

BF16 = ml_dtypes.bfloat16
F32 = mybir.dt.float32
BF = mybir.dt.bfloat16

NCORE = 8
B, N, C, H, HD = 4, 2048, 1024, 16, 64
T = B * N                 # 8192 tokens
HL = H // NCORE           # 2 heads per core
QKCH = HL * HD            # 128 q (or k) channels per core
TSLICE = T // NCORE       # 1024 tokens per core for the output projection
EPS_ROW = 64.0 * 1e-6     # eps folded into the sumsq matvec via an extra row
TOKC = 1024               # qkv/norm token chunk
QC = 1024                 # attention q chunk
KT = N // 128             # k tiles per batch (16)

_BUILD_CACHE = {}


def _build():
    if "nc" in _BUILD_CACHE:
        return _BUILD_CACHE["nc"]
    nc = bacc.Bacc(None, target_bir_lowering=False, debug=True)

    xT_d = nc.declare_dram_parameter("xT", [C, T], BF, isOutput=False)
    wqkvT_d = nc.declare_dram_parameter("wqkvT", [C, 3 * QKCH], BF, isOutput=False)
    wpT_d = nc.declare_dram_parameter("wpT", [C, C], BF, isOutput=False)
    bp_d = nc.declare_dram_parameter("bp", [1, C], BF, isOutput=False)
    cosq_d = nc.declare_dram_parameter("cosq", [HD, N], BF, isOutput=False)
    sinq_d = nc.declare_dram_parameter("sinq", [HD, N], BF, isOutput=False)
    cosk_d = nc.declare_dram_parameter("cosk", [HD, N], BF, isOutput=False)
    sink_d = nc.declare_dram_parameter("sink", [HD, N], BF, isOutput=False)
    out_d = nc.declare_dram_parameter("out", [TSLICE, C], mybir.dt.float32, isOutput=True)

    a2a_in = [nc.dram_tensor(f"a2a_in{hl}", [NCORE, HD, TSLICE], BF)
              for hl in range(HL)]
    a2a_out = [nc.dram_tensor(f"a2a_out{hl}", [NCORE, HD, TSLICE], BF)
               for hl in range(HL)]

    h2 = HD // 2

    with tile.TileContext(nc) as tc:
        with (
            tc.tile_pool(name="persist", bufs=1) as pp,
            tc.tile_pool(name="xt", bufs=3) as xtp,
            tc.tile_pool(name="nrm", bufs=2) as nrm,
            tc.tile_pool(name="nrm1", bufs=2) as nrm1,
            tc.tile_pool(name="att", bufs=3) as att,
            tc.tile_pool(name="attn1", bufs=2) as attn1,
            tc.tile_pool(name="yp", bufs=2) as yp,
            tc.tile_pool(name="pbig", bufs=2, space="PSUM") as pbig,
            tc.tile_pool(name="pacc", bufs=1, space="PSUM") as pacc,
            tc.tile_pool(name="psml", bufs=2, space="PSUM") as psml,
        ):
            # ---- resident tiles ----
            w_sb = pp.tile([128, 8, 3 * QKCH], BF)      # wqkvT, c-tiled
            bp_sb = pp.tile([1, C], BF)
            # rope tables duplicated on partitions 64:128 so hl=1 slices can
            # be used in-place (DVE needs equal input base partitions)
            rope_sb = pp.tile([128, 4, N], BF)          # cosq|sinq|cosk|sink
            qstore = pp.tile([QKCH, T], BF)
            kstore = pp.tile([QKCH, T], BF)
            # partition-swapped copies: the score matmuls alternate the PE
            # array row-halves per k-tile so LDWEIGHTS overlaps MATMUL
            q2store = pp.tile([QKCH, T], BF)
            k2store = pp.tile([QKCH, T], BF)
            vstore = pp.tile([128, T // 128, 2 * (HD + 1)], BF)
            shard = pp.tile([QKCH, T], BF)              # normalized out^T shard
            rkcol = pp.tile([128, HL, T // 128], F32)   # 8*r_k, column layout
            ones128c = pp.tile([128, 1], BF)
            ones1_64 = pp.tile([1, HD], BF)
            ones1_128 = pp.tile([1, 128], BF)
            eights1 = pp.tile([1, 1], BF)
            sqA = pp.tile([128, TOKC], BF)              # manual double buffer
            sqB = pp.tile([128, TOKC], BF)
            rr2_a = pp.tile([1, TOKC], BF)
            rr2_b = pp.tile([1, TOKC], BF)
            rr2_c = pp.tile([1, TOKC], BF)
            rr2_d = pp.tile([1, TOKC], BF)
            rr2_4 = [rr2_a, rr2_b, rr2_c, rr2_d]

            for c in range(8):
                nc.sync.dma_start(w_sb[:, c, :], wqkvT_d[128 * c:128 * (c + 1), :])
            nc.sync.dma_start(bp_sb[:], bp_d[:])
            for i, td in enumerate((cosq_d, sinq_d, cosk_d, sink_d)):
                nc.sync.dma_start(rope_sb[0:HD, i, :], td[:])
                nc.sync.dma_start(rope_sb[HD:128, i, :], td[:])
            nc.vector.memset(ones128c[:], 1.0)
            nc.vector.memset(ones1_64[:], 1.0)
            nc.vector.memset(ones1_128[:], 1.0)
            nc.vector.memset(eights1[:], 8.0)
            nc.vector.memset(vstore[:, :, HD:HD + 1], 1.0)
            nc.vector.memset(vstore[:, :, 2 * HD + 1:2 * HD + 2], 1.0)
            sqs = (sqA, sqB)

            # chain ACT instructions in emission order (sync=False) so the
            # scheduler keeps Sqrt/Exp bursts clustered -> few table loads
            acts = []

            def _act(inst):
                acts.append(inst)
                return inst

            def qkv_qk(ti):
                tok0 = ti * TOKC
                xts = []
                for t5 in range(TOKC // 512):
                    tk0 = tok0 + t5 * 512
                    xt = xtp.tile([128, 8, 512], BF, tag="xt")
                    xts.append(xt)
                    for c in range(8):
                        nc.sync.dma_start(
                            xt[:, c, :], xT_d[128 * c:128 * (c + 1), tk0:tk0 + 512])
                    for m, store in ((0, qstore), (1, kstore)):
                        ps = psml.tile([128, 512], F32, tag="psml")
                        for c in range(8):
                            nc.tensor.matmul(
                                ps[:],
                                w_sb[:, c, m * QKCH:(m + 1) * QKCH],
                                xt[:, c, :],
                                start=(c == 0), stop=(c == 7))
                        nc.vector.tensor_copy(store[:, tk0:tk0 + 512], ps[:])
                return xts

            def qkv_v(ti, xts):
                tok0 = ti * TOKC
                for t5 in range(TOKC // 512):
                    tk0 = tok0 + t5 * 512
                    xt = xts[t5]
                    for t1 in range(4):
                        ps = psml.tile([128, 512], F32, tag="psml")
                        for c in range(8):
                            nc.tensor.matmul(
                                ps[:, 0:128],
                                xt[:, c, t1 * 128:(t1 + 1) * 128],
                                w_sb[:, c, 2 * QKCH:3 * QKCH],
                                start=(c == 0), stop=(c == 7))
                        g = (tk0 // 128) + t1
                        nc.vector.tensor_copy(
                            vstore[:, g, :].rearrange(
                                "p (a b) -> p a b", b=HD + 1)[:, :, 0:HD],
                            ps[:, 0:128].rearrange("p (a b) -> p a b", b=HD))

            def norm_pre(ti):
                tok0 = ti * TOKC
                rr2s = {}
                for m, store in ((0, qstore), (1, kstore)):
                    slf = store[:, tok0:tok0 + TOKC]     # both heads at once
                    sq2 = sqs[m]
                    nc.vector.tensor_mul(sq2[:], slf, slf)
                    for hl in range(HL):
                        r0 = HD * hl
                        rr = nrm1.tile([1, TOKC], F32, tag="rr")
                        for ch in range(TOKC // 512):
                            ps = psml.tile([128, 512], F32, tag="psml")
                            nc.tensor.matmul(
                                ps[0:1, :], ones128c[r0:r0 + HD, 0:1],
                                sq2[r0:r0 + HD, ch * 512:(ch + 1) * 512],
                                start=True, stop=True)
                            with nc.allow_low_precision(reason="rms scale"):
                                nc.vector.reciprocal_approx_fast(
                                    rr[:, ch * 512:(ch + 1) * 512], ps[0:1, :])
                        rr2 = rr2_4[m * 2 + hl]
                        rr2s[m, hl] = rr2
                        _act(nc.scalar.activation(
                            rr2[:], rr[:], mybir.ActivationFunctionType.Sqrt))
                return rr2s

            def norm_post(ti, rr2s):
                tok0 = ti * TOKC
                n0 = tok0 % N
                for m, store in ((0, qstore), (1, kstore)):
                    slf = store[:, tok0:tok0 + TOKC]
                    # rope on all 128 partitions (tables duplicated per head)
                    qrot = nrm.tile([128, TOKC], BF, tag="qrot")
                    for r0 in (0, HD):
                        nc.vector.tensor_copy(
                            qrot[r0:r0 + h2, :],
                            store[r0 + h2:r0 + HD, tok0:tok0 + TOKC])
                        nc.vector.tensor_copy(
                            qrot[r0 + h2:r0 + HD, :],
                            store[r0:r0 + h2, tok0:tok0 + TOKC])
                    cw = rope_sb[:, 2 * m, n0:n0 + TOKC]
                    sw = rope_sb[:, 2 * m + 1, n0:n0 + TOKC]
                    tms = nrm.tile([128, TOKC], BF, tag="tms")
                    nc.vector.tensor_mul(slf, slf, cw)
                    nc.vector.tensor_mul(tms[:], qrot[:], sw)
                    nc.vector.tensor_add(slf, slf, tms[:])
                    if m == 0:
                        # q: apply r (both heads per op) via K=1 broadcasts
                        for ch in range(TOKC // 512):
                            psb = psml.tile([128, 512], F32, tag="psml")
                            nc.tensor.matmul(
                                psb[0:HD, :], ones1_64[:],
                                rr2s[0, 0][:, ch * 512:(ch + 1) * 512],
                                start=True, stop=True, tile_position=(0, 0))
                            nc.tensor.matmul(
                                psb[HD:128, :], ones1_64[:],
                                rr2s[0, 1][:, ch * 512:(ch + 1) * 512],
                                start=True, stop=True, tile_position=(0, HD))
                            nc.vector.tensor_mul(
                                slf[:, ch * 512:(ch + 1) * 512],
                                slf[:, ch * 512:(ch + 1) * 512], psb[:])
                        nc.vector.tensor_copy(
                            q2store[HD:128, tok0:tok0 + TOKC],
                            store[0:HD, tok0:tok0 + TOKC])
                        nc.vector.tensor_copy(
                            q2store[0:HD, tok0:tok0 + TOKC],
                            store[HD:128, tok0:tok0 + TOKC])
                    else:
                        # k: r_k rides the exp scale; transpose 8*r_k into
                        # column layout via K=1 matmuls
                        pst = psml.tile([128, 512], F32, tag="psml")
                        for hl in range(HL):
                            for g in range(TOKC // 128):
                                nc.tensor.matmul(
                                    pst[:, hl * 8 + g:hl * 8 + g + 1],
                                    rr2s[1, hl][:, g * 128:(g + 1) * 128],
                                    eights1[:],
                                    start=True, stop=True)
                        for hl in range(HL):
                            nc.vector.tensor_copy(
                                rkcol[:, hl, tok0 // 128:tok0 // 128 + TOKC // 128],
                                pst[:, hl * 8:hl * 8 + TOKC // 128])
                        nc.vector.tensor_copy(
                            k2store[HD:128, tok0:tok0 + TOKC],
                            store[0:HD, tok0:tok0 + TOKC])
                        nc.vector.tensor_copy(
                            k2store[0:HD, tok0:tok0 + TOKC],
                            store[HD:128, tok0:tok0 + TOKC])

            def attention(hl, b):
                r0 = HD * hl
                r1 = HD - r0  # swapped-store base for odd k-tiles
                boff = b * N
                for qc in range(N // QC):
                    qoff = boff + qc * QC
                    pv = pacc.tile([HD + 1, QC], F32, tag="pacc")
                    for kt in range(KT):
                        koff = boff + kt * 128
                        sps = pbig.tile([128, QC], F32, tag="pbig")
                        if kt % 2 == 0:
                            ks, qs, base = kstore, qstore, r0
                        else:
                            ks, qs, base = k2store, q2store, r1
                        for qh in range(QC // 512):
                            nc.tensor.matmul(
                                sps[:, qh * 512:(qh + 1) * 512],
                                ks[base:base + HD, koff:koff + 128],
                                qs[base:base + HD,
                                   qoff + qh * 512:qoff + (qh + 1) * 512],
                                start=True, stop=True)
                        pt = att.tile([128, QC], BF, tag="pt")
                        _act(nc.scalar.activation(
                            pt[:], sps[:], mybir.ActivationFunctionType.Exp,
                            scale=rkcol[:, hl, koff // 128:koff // 128 + 1]))
                        for qh in range(QC // 512):
                            nc.tensor.matmul(
                                pv[:, qh * 512:(qh + 1) * 512],
                                vstore[:, koff // 128,
                                       (HD + 1) * hl:(HD + 1) * (hl + 1)],
                                pt[:, qh * 512:(qh + 1) * 512],
                                start=(kt == 0), stop=(kt == KT - 1))
                    # custom-DVE recip mishandles base-partition-64 inputs;
                    # stage the denominator row at base 0 first
                    den0 = attn1.tile([1, QC], F32, tag="den0")
                    nc.vector.tensor_copy(den0[:], pv[HD:HD + 1, :])
                    drec = attn1.tile([1, QC], F32, tag="drec")
                    with nc.allow_low_precision(reason="softmax denom"):
                        nc.vector.reciprocal_approx_fast(drec[:], den0[:])
                    drecb = attn1.tile([1, QC], BF, tag="drecb")
                    nc.vector.tensor_copy(drecb[:], drec[:])
                    pvs = attn1.tile([HD, QC], BF, tag="pvs")
                    nc.vector.tensor_copy(pvs[:], pv[0:HD, :])
                    for q5 in range(QC // 512):
                        dbc = psml.tile([128, 512], F32, tag="psml")
                        nc.tensor.matmul(
                            dbc[0:HD, :], ones1_64[:],
                            drecb[:, q5 * 512:(q5 + 1) * 512],
                            start=True, stop=True)
                        nc.vector.tensor_mul(
                            shard[r0:r0 + HD,
                                  qoff + q5 * 512:qoff + (q5 + 1) * 512],
                            pvs[:, q5 * 512:(q5 + 1) * 512], dbc[0:HD, :])

            def reshard(hl):
                # AllToAll this head-half: heads -> token slices
                for j in range(NCORE):
                    nc.sync.dma_start(
                        a2a_in[hl][j],
                        shard[HD * hl:HD * (hl + 1), TSLICE * j:TSLICE * (j + 1)])
                nc.gpsimd.collective_compute(
                    "AllToAll",
                    mybir.AluOpType.bypass,
                    replica_groups=[list(range(NCORE))],
                    ins=[a2a_in[hl][:]],
                    outs=[a2a_out[hl][:]],
                )

            # ---- interleaved pipeline: qkv/norm + first-head attention ----
            for b in range(B):
                for ti in (2 * b, 2 * b + 1):
                    xts = qkv_qk(ti)
                    qkv_v(ti, xts)
                    rr2s = norm_pre(ti)
                    norm_post(ti, rr2s)
                attention(0, b)
            reshard(0)
            # second-head attention overlaps the first AllToAll
            for b in range(B):
                attention(1, b)
            reshard(1)

            # ---- output projection on this core's token slice ----
            wp1 = xtp.tile([128, 8, 512], BF, tag="xt")
            wp2 = xtp.tile([128, 8, 512], BF, tag="xt")
            for c in range(8):
                nc.sync.dma_start(wp1[:, c, :], wpT_d[128 * c:128 * (c + 1), 0:512])
                nc.sync.dma_start(wp2[:, c, :], wpT_d[128 * c:128 * (c + 1), 512:1024])
            wps = (wp1, wp2)
            for th in range(2):  # token halves of this core's slice
                gat = xtp.tile([128, 8, 512], BF, tag="xt")
                for c in range(8):
                    nc.sync.dma_start(gat[0:HD, c, :],
                                      a2a_out[0][c, :, th * 512:(th + 1) * 512])
                    nc.sync.dma_start(gat[HD:128, c, :],
                                      a2a_out[1][c, :, th * 512:(th + 1) * 512])
                for t1 in range(4):
                    for d5 in range(2):
                        if (t1 * 2 + d5) % 2 == 0:
                            ps = psml.tile([128, 512], F32, tag="psml")
                        else:
                            psw = pbig.tile([128, 1024], F32, tag="pbig")
                            ps = psw[:, 0:512]
                        for c in range(8):
                            nc.tensor.matmul(
                                ps[:],
                                gat[:, c, t1 * 128:(t1 + 1) * 128],
                                wps[d5][:, c, :],
                                start=(c == 0), stop=False)
                        nc.tensor.matmul(
                            ps[:], ones1_128[:], bp_sb[:, d5 * 512:(d5 + 1) * 512],
                            start=False, stop=True)
                        ysb = yp.tile([128, 512], F32, tag="ysb")
                        nc.vector.tensor_copy(ysb[:], ps[:])
                        nc.sync.dma_start(
                            out_d[th * 512 + t1 * 128:th * 512 + (t1 + 1) * 128,
                                  d5 * 512:(d5 + 1) * 512],
                            ysb[:])

        if os.environ.get("ACT_CHAIN", "0") == "1":
            for a, b2 in zip(acts, acts[1:]):
                tile.add_dep_helper(b2.ins, a.ins, sync=False, reason="act table cluster")

    nc.compile()
    _BUILD_CACHE["nc"] = nc
    return nc


def _host_prep(x, rope_cos, rope_sin, w_qkv, w_proj, b_proj, q_norm_w, k_norm_w):
    x = np.asarray(x, np.float32)
    xT = np.ascontiguousarray(x.reshape(T, C).T).astype(BF16)
    cosT = np.asarray(rope_cos, np.float32)[0, 0].T          # [hd, N]
    sinT = np.asarray(rope_sin, np.float32)[0, 0].T

    def fold(w):
        w = np.asarray(w, np.float32)
        cw = (cosT * w[:, None]).astype(BF16)
        sw = np.empty_like(sinT)
        sw[:32] = -sinT[:32] * w[32:64, None]
        sw[32:] = sinT[32:] * w[0:32, None]
        return cw, sw.astype(BF16)

    cosq, sinq = fold(q_norm_w)
    cosk, sink = fold(k_norm_w)
    wpT = np.ascontiguousarray(np.asarray(w_proj, np.float32).T).astype(BF16)
    bp = np.asarray(b_proj, np.float32).reshape(1, C).astype(BF16)
    w_qkv = np.asarray(w_qkv, np.float32)

    in_maps = []
    for r in range(NCORE):
        wq = w_qkv[QKCH * r:QKCH * (r + 1), :].T
        wk = w_qkv[C + QKCH * r:C + QKCH * (r + 1), :].T
        wv = w_qkv[2 * C + QKCH * r:2 * C + QKCH * (r + 1), :].T
        wqkvT = np.ascontiguousarray(
            np.concatenate([wq, wk, wv], axis=1)).astype(BF16)
        in_maps.append({
            "xT": xT, "wqkvT": wqkvT, "wpT": wpT, "bp": bp,
            "cosq": cosq, "sinq": sinq, "cosk": cosk, "sink": sink,
        })
    return in_maps


def _run(in_maps, trace=False, **kwargs):
    nc = _build()
    return run_bass_kernel_spmd(
        nc, in_maps, core_ids=list(range(NCORE)), trace=trace, **kwargs)


def kernel(**inputs):
    in_maps = _host_prep(**inputs)
    res = _run(in_maps)
    y = np.concatenate(
        [np.asarray(res.results[r]["out"], np.float32) for r in range(NCORE)],
        axis=0)
    return y.reshape(B, N, C)



# revision 8
# speedup vs baseline: 1.2461x; 1.2461x over previous
"""Trainium2 Bass kernel for nn_Attention_22539988369511 (v2).

Dense transformer attention block (B=4, N=2048, C=1024, H=16, hd=64),
sharded over 8 NeuronCores with tensor parallelism over heads (2 heads
per core), per-batch AllToAll with token striping for the output
projection.

Key design points vs v1:
 - Scores for the two heads run CONCURRENTLY in disjoint PE row-groups
   (K=64 each: rows 0-63 and 64-127), writing adjacent PSUM banks, so
   one exp activation covers both heads ([128, 1024] per k-tile).
 - All scale factors fold away: q tables carry w_q (8*0.125=1), k
   tables carry 8*w_k, and the per-token rsqrt factors are multiplied
   into qstore/kstore via K=1 broadcast matmuls, so exp has no scale AP.
 - RMS rsqrt = exp(-0.5*ln(sumsq+eps)) on ACT: ln+exp live in ONE
   activation table set -> zero table switches in the whole kernel.
   Sumsq lands at partitions {0,32,64,96} via mask matmuls so the K=1
   broadcast matmuls stay legal.
 - qkv/norm work for batch b+1 and projection for b-1 are interleaved
   into attention(b)'s PE-queue gaps (attention is ACT-bound).
 - Per-batch AllToAll (token stripes of 256) overlaps collectives with
   compute; each core owns stripe j of every batch for the projection.
"""
import os
import sys

import numpy as np
import ml_dtypes

for _p in ("/opt/trn_rl_repo", "/root/.axon_site/_ro/trn_rl_repo"):
    if os.path.isdir(_p) and _p not in sys.path:
        sys.path.append(_p)

import concourse.bass as bass
import concourse.mybir as mybir
from concourse import bacc, tile
from concourse.bass_utils import run_bass_kernel_spmd

BF16 = ml_dtypes.bfloat16
F32 = mybir.dt.float32
BF = mybir.dt.bfloat16
AF = mybir.ActivationFunctionType

NCORE = 8
B, N, C, H, HD = 4, 2048, 1024, 16, 64
HL = H // NCORE           # 2 heads per core
QKCH = HL * HD            # 128 q (or k) channels per core
T = B * N                 # 8192 tokens
ST = N // NCORE           # 256-token output stripe per core per batch
TOKC = 1024               # qkv/norm token chunk
QC = 512                  # attention q chunk
KT = N // 128             # k tiles per batch (16)
h2 = HD // 2
EPS_SUM = 64.0 * 1e-6     # eps on the 64-element sumsq

_BUILD_CACHE = {}


def _build():
    if "nc" in _BUILD_CACHE:
        return _BUILD_CACHE["nc"]
    nc = bacc.Bacc(None, target_bir_lowering=False, debug=True)

    xT_d = nc.declare_dram_parameter("xT", [128, 8, T], BF, isOutput=False)
    wqkv_d = nc.declare_dram_parameter("wqkv", [128, 8, 3 * QKCH], BF, isOutput=False)
    wp_d = nc.declare_dram_parameter("wp", [128, 8, C], BF, isOutput=False)
    bp_d = nc.declare_dram_parameter("bp", [1, C], BF, isOutput=False)
    cosq_d = nc.declare_dram_parameter("cosq", [128, N], BF, isOutput=False)
    sinq_d = nc.declare_dram_parameter("sinq", [128, N], BF, isOutput=False)
    cosk_d = nc.declare_dram_parameter("cosk", [128, N], BF, isOutput=False)
    sink_d = nc.declare_dram_parameter("sink", [128, N], BF, isOutput=False)
    out_d = nc.declare_dram_parameter("out", [B, ST, C], F32, isOutput=True)

    a2a_in = [nc.dram_tensor(f"a2a_in{b}", [NCORE, QKCH, ST], BF) for b in range(B)]
    a2a_out = [nc.dram_tensor(f"a2a_out{b}", [NCORE, QKCH, ST], BF) for b in range(B)]

    with tile.TileContext(nc) as tc:
        with (
            tc.tile_pool(name="persist", bufs=1) as pp,
            tc.tile_pool(name="xt", bufs=3) as xtp,
            tc.tile_pool(name="nrm", bufs=2) as nrm,
            tc.tile_pool(name="att", bufs=2) as att,
            tc.tile_pool(name="shp", bufs=2) as shp,
            tc.tile_pool(name="gatp", bufs=2) as gatp,
            tc.tile_pool(name="drp", bufs=2) as drp,
            tc.tile_pool(name="scp", bufs=2, space="PSUM") as scp,
            tc.tile_pool(name="paccp", bufs=2, space="PSUM") as paccp,
            tc.tile_pool(name="psml", bufs=2, space="PSUM") as psml,
        ):
            # ---- resident tiles ----
            w_sb = pp.tile([128, 8, 3 * QKCH], BF)
            wp_sb = pp.tile([128, 8, C], BF)
            bp_sb = pp.tile([1, C], BF)
            rope_sb = pp.tile([128, 4, N], BF)          # cosq|sinq|cosk|sink
            qstore = pp.tile([QKCH, T], BF)
            kstore = pp.tile([QKCH, T], BF)
            vstore = pp.tile([128, T // 128, 2 * (HD + 1)], BF)
            maskA = pp.tile([128, 97], BF)              # q sumsq -> rows 0, 32
            maskB = pp.tile([128, 97], BF)              # k sumsq -> rows 64, 96
            ones_sb = pp.tile([128, HD], BF)            # K=1 lhsT rows at any partition
            ones1_128 = pp.tile([1, 128], BF)
            eps_col = pp.tile([128, 1], F32)            # rms eps as activation bias

            nc.sync.dma_start(w_sb[:], wqkv_d[:])
            nc.sync.dma_start(wp_sb[:], wp_d[:])
            nc.sync.dma_start(bp_sb[:], bp_d[:])
            for i, td in enumerate((cosq_d, sinq_d, cosk_d, sink_d)):
                nc.sync.dma_start(rope_sb[:, i, :], td[:])
            nc.vector.memset(maskA[:], 0.0)
            nc.vector.memset(maskB[:], 0.0)
            nc.vector.memset(maskA[0:64, 0:1], 1.0)
            nc.vector.memset(maskA[64:128, 32:33], 1.0)
            nc.vector.memset(maskB[0:64, 64:65], 1.0)
            nc.vector.memset(maskB[64:128, 96:97], 1.0)
            nc.vector.memset(ones_sb[:], 1.0)
            nc.vector.memset(ones1_128[:], 1.0)
            nc.vector.memset(eps_col[:], EPS_SUM)
            nc.vector.memset(vstore[:, :, HD:HD + 1], 1.0)
            nc.vector.memset(vstore[:, :, 2 * HD + 1:2 * HD + 2], 1.0)

            # ---------- qkv + norm for one 1024-token chunk ----------
            def qkv_norm_items(b):
                items = []
                for ti in (2 * b, 2 * b + 1):
                    tok0 = ti * TOKC
                    n0 = tok0 % N
                    st = {}

                    def qk_group(t5, m, ti=ti, tok0=tok0, st=st):
                        tk0 = tok0 + t5 * 512
                        if m == 0:
                            xt = xtp.tile([128, 8, 512], BF, tag="xt")
                            nc.sync.dma_start(xt[:], xT_d[:, :, tk0:tk0 + 512])
                            st[t5] = xt
                        xt = st[t5]
                        store = qstore if m == 0 else kstore
                        ps = psml.tile([128, 512], F32, tag="psml")
                        for c in range(8):
                            nc.tensor.matmul(
                                ps[:], w_sb[:, c, m * QKCH:(m + 1) * QKCH],
                                xt[:, c, :], start=(c == 0), stop=(c == 7))
                        nc.vector.tensor_copy(store[:, tk0:tk0 + 512], ps[:])

                    def v_group(t5, t1pair, tok0=tok0, st=st):
                        tk0 = tok0 + t5 * 512
                        xt = st[t5]
                        for t1 in (2 * t1pair, 2 * t1pair + 1):
                            ps = psml.tile([128, 512], F32, tag="psml")
                            for c in range(8):
                                nc.tensor.matmul(
                                    ps[:, 0:128],
                                    xt[:, c, t1 * 128:(t1 + 1) * 128],
                                    w_sb[:, c, 2 * QKCH:3 * QKCH],
                                    start=(c == 0), stop=(c == 7))
                            g = (tk0 // 128) + t1
                            nc.vector.tensor_copy(
                                vstore[:, g, :].rearrange(
                                    "p (a b) -> p a b", b=HD + 1)[:, :, 0:HD],
                                ps[:, 0:128].rearrange("p (a b) -> p a b", b=HD))

                    def squares(tok0=tok0, st=st):
                        sqq = nrm.tile([128, TOKC], BF, tag="sqq")
                        sqk = nrm.tile([128, TOKC], BF, tag="sqk")
                        nc.vector.tensor_mul(
                            sqq[:], qstore[:, tok0:tok0 + TOKC], qstore[:, tok0:tok0 + TOKC])
                        nc.vector.tensor_mul(
                            sqk[:], kstore[:, tok0:tok0 + TOKC], kstore[:, tok0:tok0 + TOKC])
                        st["sqq"], st["sqk"] = sqq, sqk

                    def rfactor(ch, st=st):
                        ps = psml.tile([128, 512], F32, tag="psml")
                        nc.tensor.matmul(
                            ps[0:97, :], maskA[:],
                            st["sqq"][:, ch * 512:(ch + 1) * 512],
                            start=True, stop=False)
                        nc.tensor.matmul(
                            ps[0:97, :], maskB[:],
                            st["sqk"][:, ch * 512:(ch + 1) * 512],
                            start=False, stop=True)
                        lnscr = nrm.tile([97, 512], F32, tag="lnscr")
                        nc.scalar.activation(
                            lnscr[:], ps[0:97, :], AF.Ln, bias=eps_col[0:97, :])
                        rall = nrm.tile([97, 512], BF, tag="rall")
                        nc.scalar.activation(rall[:], lnscr[:], AF.Exp, scale=-0.5)
                        st["rall%d" % ch] = rall

                    def rope(m, tok0=tok0, n0=n0):
                        store = qstore if m == 0 else kstore
                        slf = store[:, tok0:tok0 + TOKC]
                        qrot = nrm.tile([128, TOKC], BF, tag="qrot")
                        for r0 in (0, HD):
                            nc.vector.tensor_copy(
                                qrot[r0:r0 + h2, :], store[r0 + h2:r0 + HD, tok0:tok0 + TOKC])
                            nc.vector.tensor_copy(
                                qrot[r0 + h2:r0 + HD, :], store[r0:r0 + h2, tok0:tok0 + TOKC])
                        cw = rope_sb[:, 2 * m, n0:n0 + TOKC]
                        sw = rope_sb[:, 2 * m + 1, n0:n0 + TOKC]
                        tms = nrm.tile([128, TOKC], BF, tag="tms")
                        nc.vector.tensor_mul(slf, slf, cw)
                        nc.vector.tensor_mul(tms[:], qrot[:], sw)
                        nc.vector.tensor_add(slf, slf, tms[:])

                    def rmul(m, tok0=tok0, st=st):
                        # multiply per-token rsqrt into the store (both heads)
                        store = qstore if m == 0 else kstore
                        rows = (0, 32) if m == 0 else (64, 96)
                        for ch in range(2):
                            rall = st["rall%d" % ch]
                            bc = psml.tile([128, 512], F32, tag="psml")
                            nc.tensor.matmul(
                                bc[0:64, :], ones_sb[rows[0]:rows[0] + 1, :],
                                rall[rows[0]:rows[0] + 1, :],
                                start=True, stop=True, tile_position=(rows[0], 0))
                            nc.tensor.matmul(
                                bc[64:128, :], ones_sb[rows[1]:rows[1] + 1, :],
                                rall[rows[1]:rows[1] + 1, :],
                                start=True, stop=True, tile_position=(rows[1], 64))
                            sl = store[:, tok0 + ch * 512:tok0 + (ch + 1) * 512]
                            nc.vector.tensor_mul(sl, sl, bc[:])

                    items += [
                        lambda f=qk_group: (f(0, 0), f(0, 1)),
                        lambda f=v_group: f(0, 0),
                        lambda f=v_group: f(0, 1),
                        lambda f=qk_group: (f(1, 0), f(1, 1)),
                        lambda f=v_group: f(1, 0),
                        lambda f=v_group: f(1, 1),
                        squares,
                        lambda f=rfactor: f(0),
                        lambda f=rfactor: f(1),
                        lambda f=rope: f(0),
                        lambda f=rmul: f(0),
                        lambda f=rope: f(1),
                        lambda f=rmul: f(1),
                    ]
                return items

            # ---------- attention for one batch ----------
            def attention(b, feeder):
                boff = b * N
                shard = shp.tile([QKCH, N], BF, tag="shard")
                fi = 0
                slot = 0
                prev = [None]

                def drain1(pr):
                    p0, p1, qc = pr
                    den0 = drp.tile([1, QC], F32, tag="den0")
                    den1 = drp.tile([1, QC], F32, tag="den1")
                    nc.vector.tensor_copy(den0[:], p0[64:65, :])
                    nc.vector.tensor_copy(den1[:], p1[64:65, :])
                    pvs = drp.tile([128, QC], BF, tag="pvs")
                    nc.vector.tensor_copy(pvs[0:64, :], p0[0:64, :])
                    nc.vector.tensor_copy(pvs[64:128, :], p1[0:64, :])
                    pr += [den0, den1, pvs]

                def drain2(pr):
                    den0, den1 = pr[3], pr[4]
                    drec0 = drp.tile([1, QC], F32, tag="drec0")
                    drec1 = drp.tile([1, QC], F32, tag="drec1")
                    with nc.allow_low_precision(reason="softmax denom"):
                        nc.vector.reciprocal_approx_fast(drec0[:], den0[:])
                        nc.vector.reciprocal_approx_fast(drec1[:], den1[:])
                    d0 = drp.tile([1, QC], BF, tag="d0")
                    d1 = drp.tile([1, QC], BF, tag="d1")
                    nc.vector.tensor_copy(d0[:], drec0[:])
                    nc.vector.tensor_copy(d1[:], drec1[:])
                    pr += [d0, d1]

                def drain3(pr):
                    _p0, _p1, qc, _d0f, _d1f, pvs, d0, d1 = pr
                    dbc = psml.tile([128, 512], F32, tag="psml")
                    nc.tensor.matmul(dbc[0:64, :], ones_sb[0:1, :], d0[:],
                                     start=True, stop=True, tile_position=(0, 0))
                    nc.tensor.matmul(dbc[64:128, :], ones_sb[0:1, :], d1[:],
                                     start=True, stop=True, tile_position=(0, 64))
                    nc.vector.tensor_mul(
                        shard[:, qc * QC:(qc + 1) * QC], pvs[:], dbc[:])

                for qc in range(N // QC):
                    qoff = boff + qc * QC
                    p0 = paccp.tile([HD + 1, QC], F32, tag="pacc")
                    p1 = paccp.tile([HD + 1, QC], F32, tag="pacc")
                    last_pt = None
                    for kt in range(KT):
                        koff = boff + kt * 128
                        g = koff // 128
                        sc = scp.tile([128, 2, QC], F32, tag="sc")
                        nc.tensor.matmul(
                            sc[:, 0, :], kstore[0:HD, koff:koff + 128],
                            qstore[0:HD, qoff:qoff + QC], start=True, stop=True)
                        nc.tensor.matmul(
                            sc[:, 1, :], kstore[HD:128, koff:koff + 128],
                            qstore[HD:128, qoff:qoff + QC], start=True, stop=True)
                        if prev[0] is not None:
                            if kt == 0:
                                drain1(prev[0])
                            elif kt == 1:
                                drain2(prev[0])
                            elif kt == 2:
                                drain3(prev[0])
                                prev[0] = None
                        pt = att.tile([128, 2, QC], BF, tag="pt")
                        nc.scalar.activation(pt[:], sc[:], AF.Exp)
                        if fi < len(feeder) and feeder[fi][0] <= slot:
                            feeder[fi][1]()
                            fi += 1
                        if last_pt is not None:
                            gp, ptp = last_pt
                            nc.tensor.matmul(p0[:], vstore[:, gp, 0:HD + 1],
                                             ptp[:, 0, :], start=(gp % KT == 0), stop=False)
                            nc.tensor.matmul(p1[:], vstore[:, gp, HD + 1:2 * (HD + 1)],
                                             ptp[:, 1, :], start=(gp % KT == 0), stop=False)
                        last_pt = (g, pt)
                        slot += 1
                    gp, ptp = last_pt
                    nc.tensor.matmul(p0[:], vstore[:, gp, 0:HD + 1],
                                     ptp[:, 0, :], start=False, stop=True)
                    nc.tensor.matmul(p1[:], vstore[:, gp, HD + 1:2 * (HD + 1)],
                                     ptp[:, 1, :], start=False, stop=True)
                    prev[0] = [p0, p1, qc]
                # flush remaining feeder + final drain
                while fi < len(feeder):
                    feeder[fi][1]()
                    fi += 1
                drain1(prev[0])
                drain2(prev[0])
                drain3(prev[0])
                return shard

            def reshard(b, shard):
                nc.sync.dma_start(
                    a2a_in[b][:].rearrange("j p t -> p j t"),
                    shard[:].rearrange("p (j t) -> p j t", j=NCORE))
                nc.gpsimd.collective_compute(
                    "AllToAll",
                    mybir.AluOpType.bypass,
                    replica_groups=[list(range(NCORE))],
                    ins=[a2a_in[b][:]],
                    outs=[a2a_out[b][:]],
                )

            def proj_items(b):
                st = {}

                def gather():
                    gat = gatp.tile([128, 8, ST], BF, tag="gat")
                    nc.gpsimd.dma_start(
                        gat[:], a2a_out[b][:].rearrange("c p t -> p c t"))
                    st["g"] = gat

                def mmgroup(tg, d5):
                    gat = st["g"]
                    ps = psml.tile([128, 512], F32, tag="psml")
                    for c in range(8):
                        nc.tensor.matmul(
                            ps[:], gat[:, c, tg * 128:(tg + 1) * 128],
                            wp_sb[:, c, d5 * 512:(d5 + 1) * 512],
                            start=(c == 0), stop=False)
                    nc.tensor.matmul(
                        ps[:], ones1_128[:], bp_sb[:, d5 * 512:(d5 + 1) * 512],
                        start=False, stop=True)
                    ysb = gatp.tile([128, 512], F32, tag="ysb")
                    nc.vector.tensor_copy(ysb[:], ps[:])
                    nc.gpsimd.dma_start(
                        out_d[b, tg * 128:(tg + 1) * 128, d5 * 512:(d5 + 1) * 512],
                        ysb[:])

                return [gather] + [
                    (lambda tg=tg, d5=d5: mmgroup(tg, d5))
                    for tg in range(2) for d5 in range(2)]

            # ---------- main pipeline ----------
            for it in qkv_norm_items(0):
                it()
            for b in range(B):
                feeder = []
                if b + 1 < B:
                    feeder += [(i, f) for i, f in enumerate(qkv_norm_items(b + 1))]
                if b >= 1:
                    feeder += [(28 + i, f) for i, f in enumerate(proj_items(b - 1))]
                feeder.sort(key=lambda x: x[0])
                shard = attention(b, feeder)
                reshard(b, shard)
            for f in proj_items(B - 1):
                f()

    nc.compile()
    _BUILD_CACHE["nc"] = nc
    return nc


def _host_prep(x, rope_cos, rope_sin, w_qkv, w_proj, b_proj, q_norm_w, k_norm_w):
    x = np.asarray(x, np.float32)
    xT = np.ascontiguousarray(
        x.reshape(T, C).T.reshape(8, 128, T).transpose(1, 0, 2)).astype(BF16)
    cosT = np.asarray(rope_cos, np.float32)[0, 0].T          # [hd, N]
    sinT = np.asarray(rope_sin, np.float32)[0, 0].T

    def fold(w, s):
        w = np.asarray(w, np.float32)
        cw = cosT * w[:, None] * s
        sw = np.empty_like(sinT)
        sw[:h2] = -sinT[:h2] * w[h2:HD, None] * s
        sw[h2:] = sinT[h2:] * w[0:h2, None] * s
        dup = lambda a: np.ascontiguousarray(np.concatenate([a, a], 0)).astype(BF16)
        return dup(cw), dup(sw)

    cosq, sinq = fold(q_norm_w, 1.0)     # 8 (rms) * 0.125 (softmax scale) = 1
    cosk, sink = fold(k_norm_w, 8.0)
    w_proj = np.asarray(w_proj, np.float32)
    wp = np.ascontiguousarray(
        w_proj.T.reshape(8, 128, C).transpose(1, 0, 2)).astype(BF16)
    bp = np.asarray(b_proj, np.float32).reshape(1, C).astype(BF16)
    w_qkv = np.asarray(w_qkv, np.float32)

    in_maps = []
    for r in range(NCORE):
        wq = w_qkv[QKCH * r:QKCH * (r + 1), :].T
        wk = w_qkv[C + QKCH * r:C + QKCH * (r + 1), :].T
        wv = w_qkv[2 * C + QKCH * r:2 * C + QKCH * (r + 1), :].T
        wqkvT = np.concatenate([wq, wk, wv], axis=1)         # [C, 384]
        wqkv = np.ascontiguousarray(
            wqkvT.reshape(8, 128, 3 * QKCH).transpose(1, 0, 2)).astype(BF16)
        in_maps.append({
            "xT": xT, "wqkv": wqkv, "wp": wp, "bp": bp,
            "cosq": cosq, "sinq": sinq, "cosk": cosk, "sink": sink,
        })
    return in_maps


def _run(in_maps, trace=False, **kwargs):
    nc = _build()
    return run_bass_kernel_spmd(
        nc, in_maps, core_ids=list(range(NCORE)), trace=trace, **kwargs)


def _unshard(res):
    outs = np.stack(
        [np.asarray(res.results[r]["out"], np.float32) for r in range(NCORE)])
    # outs: [core j, b, 256, C] -> y[b, j*256:(j+1)*256, :]
    return np.ascontiguousarray(outs.transpose(1, 0, 2, 3).reshape(B, N, C))


def kernel(**inputs):
    in_maps = _host_prep(**inputs)
    res = _run(in_maps)
    return _unshard(res)


# revision 22
# speedup vs baseline: 1.3176x; 1.0574x over previous
"""Trainium2 Bass kernel for nn_Attention_22539988369511 (v2).

Dense transformer attention block (B=4, N=2048, C=1024, H=16, hd=64),
sharded over 8 NeuronCores with tensor parallelism over heads (2 heads
per core), per-batch AllToAll with token striping for the output
projection.

Key design points vs v1:
 - Scores for the two heads run CONCURRENTLY in disjoint PE row-groups
   (K=64 each: rows 0-63 and 64-127), writing adjacent PSUM banks, so
   one exp activation covers both heads ([128, 1024] per k-tile).
 - All scale factors fold away: q tables carry w_q (8*0.125=1), k
   tables carry 8*w_k, and the per-token rsqrt factors are multiplied
   into qstore/kstore via K=1 broadcast matmuls, so exp has no scale AP.
 - RMS rsqrt = exp(-0.5*ln(sumsq+eps)) on ACT: ln+exp live in ONE
   activation table set -> zero table switches in the whole kernel.
   Sumsq lands at partitions {0,32,64,96} via mask matmuls so the K=1
   broadcast matmuls stay legal.
 - qkv/norm work for batch b+1 and projection for b-1 are interleaved
   into attention(b)'s PE-queue gaps (attention is ACT-bound).
 - Per-batch AllToAll (token stripes of 256) overlaps collectives with
   compute; each core owns stripe j of every batch for the projection.
"""
import os
import sys

import numpy as np
import ml_dtypes

for _p in ("/opt/trn_rl_repo", "/root/.axon_site/_ro/trn_rl_repo"):
    if os.path.isdir(_p) and _p not in sys.path:
        sys.path.append(_p)

import concourse.bass as bass
import concourse.mybir as mybir
from concourse import bacc, tile
from concourse.bass_utils import run_bass_kernel_spmd

BF16 = ml_dtypes.bfloat16
F32 = mybir.dt.float32
BF = mybir.dt.bfloat16
AF = mybir.ActivationFunctionType

NCORE = 8
B, N, C, H, HD = 4, 2048, 1024, 16, 64
HL = H // NCORE           # 2 heads per core
QKCH = HL * HD            # 128 q (or k) channels per core
T = B * N                 # 8192 tokens
ST = N // NCORE           # 256-token output stripe per core per batch
TOKC = 1024               # qkv/norm token chunk
QC = 512                  # attention q chunk
KT = N // 128             # k tiles per batch (16)
h2 = HD // 2
EPS_SUM = 64.0 * 1e-6     # eps on the 64-element sumsq

_BUILD_CACHE = {}


def _build():
    if "nc" in _BUILD_CACHE:
        return _BUILD_CACHE["nc"]
    nc = bacc.Bacc(None, target_bir_lowering=False, debug=True)

    xT_d = nc.declare_dram_parameter("xT", [128, 8, T], BF, isOutput=False)
    wqkv_d = nc.declare_dram_parameter("wqkv", [128, 8, 3 * QKCH], BF, isOutput=False)
    wp_d = nc.declare_dram_parameter("wp", [128, 8, C], BF, isOutput=False)
    bp_d = nc.declare_dram_parameter("bp", [1, C], BF, isOutput=False)
    cosq_d = nc.declare_dram_parameter("cosq", [128, N], BF, isOutput=False)
    sinq_d = nc.declare_dram_parameter("sinq", [128, N], BF, isOutput=False)
    cosk_d = nc.declare_dram_parameter("cosk", [128, N], BF, isOutput=False)
    sink_d = nc.declare_dram_parameter("sink", [128, N], BF, isOutput=False)
    out_d = nc.declare_dram_parameter("out", [B, ST, C], F32, isOutput=True)

    a2a_in = [nc.dram_tensor(f"a2a_in{b}", [NCORE, QKCH, ST], BF) for b in range(B)]
    a2a_out = [nc.dram_tensor(f"a2a_out{b}", [NCORE, QKCH, ST], BF) for b in range(B)]

    with tile.TileContext(nc) as tc:
        with (
            tc.tile_pool(name="persist", bufs=1) as pp,
            tc.tile_pool(name="xt", bufs=3) as xtp,
            tc.tile_pool(name="nrm", bufs=2) as nrm,
            tc.tile_pool(name="rfp", bufs=4) as rfp,
            tc.tile_pool(name="att", bufs=2) as att,
            tc.tile_pool(name="shp", bufs=2) as shp,
            tc.tile_pool(name="gatp", bufs=2) as gatp,
            tc.tile_pool(name="drp", bufs=2) as drp,
            tc.tile_pool(name="scp", bufs=2, space="PSUM") as scp,
            tc.tile_pool(name="paccp", bufs=2, space="PSUM") as paccp,
            tc.tile_pool(name="psml", bufs=2, space="PSUM") as psml,
        ):
            # ---- resident tiles ----
            w_sb = pp.tile([128, 8, 3 * QKCH], BF)
            wp_sb = pp.tile([128, 8, C], BF)
            bp_sb = pp.tile([1, C], BF)
            rope_sb = pp.tile([128, 4, N], BF)          # cosq|sinq|cosk|sink
            qstore = pp.tile([QKCH, T], BF)
            kstore = pp.tile([QKCH, T], BF)
            vstore = pp.tile([128, T // 128, 2 * (HD + 1)], BF)
            maskA = pp.tile([128, 97], BF)              # q sumsq -> rows 0, 32
            maskB = pp.tile([128, 97], BF)              # k sumsq -> rows 64, 96
            ones_sb = pp.tile([128, HD], BF)            # K=1 lhsT rows at any partition
            ones1_128 = pp.tile([1, 128], BF)
            eps_col = pp.tile([128, 1], F32)            # rms eps as activation bias

            nc.sync.dma_start(w_sb[:], wqkv_d[:])
            nc.sync.dma_start(wp_sb[:], wp_d[:])
            nc.sync.dma_start(bp_sb[:], bp_d[:])
            for i, td in enumerate((cosq_d, sinq_d, cosk_d, sink_d)):
                nc.sync.dma_start(rope_sb[:, i, :], td[:])
            nc.vector.memset(maskA[:], 0.0)
            nc.vector.memset(maskB[:], 0.0)
            nc.vector.memset(maskA[0:64, 0:1], 1.0)
            nc.vector.memset(maskA[64:128, 32:33], 1.0)
            nc.vector.memset(maskB[0:64, 64:65], 1.0)
            nc.vector.memset(maskB[64:128, 96:97], 1.0)
            nc.vector.memset(ones_sb[:], 1.0)
            nc.vector.memset(ones1_128[:], 1.0)
            nc.vector.memset(eps_col[:], EPS_SUM)
            nc.vector.memset(vstore[:, :, HD:HD + 1], 1.0)
            nc.vector.memset(vstore[:, :, 2 * HD + 1:2 * HD + 2], 1.0)

            # ---------- qkv + norm for one batch (two 1024-token chunks) ----
            # ACT-table discipline: all 4 Ln calls for the batch execute as
            # one consecutive cluster (one natural_log load), then the
            # exp(-0.5) calls rejoin the attention Exp stream (one exp load).
            # scheduler-order glue so Ln bursts stay contiguous between
            # attention Exps (minimizes ACT table-set reloads)
            sched = {"last": None, "bar": None}

            def _chain(instrs):
                for a, b2 in zip(instrs, instrs[1:]):
                    tile.add_dep_helper(
                        b2.ins, a.ins, sync=False, reason="act table cluster")

            def qkv_norm_items(b):
                items = []
                lns = []        # deferred matvec+ln closures, run as one item
                exps = []       # deferred exp(-0.5) closures
                tail = []       # rope + r-multiply items
                actins = []     # ACT instructions to keep contiguous
                for ti in (2 * b, 2 * b + 1):
                    tok0 = ti * TOKC
                    n0 = tok0 % N
                    st = {}

                    def qk_group(t5, m, ti=ti, tok0=tok0, st=st):
                        tk0 = tok0 + t5 * 512
                        if m == 0:
                            xt = xtp.tile([128, 8, 512], BF, tag="xt")
                            nc.sync.dma_start(xt[:], xT_d[:, :, tk0:tk0 + 512])
                            st[t5] = xt
                        xt = st[t5]
                        store = qstore if m == 0 else kstore
                        ps = psml.tile([128, 512], F32, tag="psml")
                        for c in range(8):
                            nc.tensor.matmul(
                                ps[:], w_sb[:, c, m * QKCH:(m + 1) * QKCH],
                                xt[:, c, :], start=(c == 0), stop=(c == 7))
                        nc.vector.tensor_copy(store[:, tk0:tk0 + 512], ps[:])

                    def v_group(t5, t1pair, tok0=tok0, st=st):
                        tk0 = tok0 + t5 * 512
                        xt = st[t5]
                        for t1 in (2 * t1pair, 2 * t1pair + 1):
                            ps = psml.tile([128, 512], F32, tag="psml")
                            for c in range(8):
                                nc.tensor.matmul(
                                    ps[:, 0:128],
                                    xt[:, c, t1 * 128:(t1 + 1) * 128],
                                    w_sb[:, c, 2 * QKCH:3 * QKCH],
                                    start=(c == 0), stop=(c == 7))
                            g = (tk0 // 128) + t1
                            nc.vector.tensor_copy(
                                vstore[:, g, :].rearrange(
                                    "p (a b) -> p a b", b=HD + 1)[:, :, 0:HD],
                                ps[:, 0:128].rearrange("p (a b) -> p a b", b=HD))

                    def squares(tok0=tok0, st=st):
                        sqq = nrm.tile([128, TOKC], BF, tag="sqq")
                        sqk = nrm.tile([128, TOKC], BF, tag="sqk")
                        nc.vector.tensor_mul(
                            sqq[:], qstore[:, tok0:tok0 + TOKC], qstore[:, tok0:tok0 + TOKC])
                        nc.vector.tensor_mul(
                            sqk[:], kstore[:, tok0:tok0 + TOKC], kstore[:, tok0:tok0 + TOKC])
                        st["sqq"], st["sqk"] = sqq, sqk

                    def rf_ln(ch, st=st):
                        ps = psml.tile([128, 512], F32, tag="psml")
                        nc.tensor.matmul(
                            ps[0:97, :], maskA[:],
                            st["sqq"][:, ch * 512:(ch + 1) * 512],
                            start=True, stop=False)
                        nc.tensor.matmul(
                            ps[0:97, :], maskB[:],
                            st["sqk"][:, ch * 512:(ch + 1) * 512],
                            start=False, stop=True)
                        lnscr = rfp.tile([97, 512], F32, tag="lnscr")
                        actins.append(nc.scalar.activation(
                            lnscr[:], ps[0:97, :], AF.Ln, bias=eps_col[0:97, :]))
                        st["ln%d" % ch] = lnscr

                    def rf_exp(ch, st=st):
                        rall = rfp.tile([97, 512], BF, tag="rall")
                        actins.append(nc.scalar.activation(
                            rall[:], st["ln%d" % ch], AF.Exp, scale=-0.5))
                        st["rall%d" % ch] = rall

                    def rope(m, tok0=tok0, n0=n0):
                        store = qstore if m == 0 else kstore
                        slf = store[:, tok0:tok0 + TOKC]
                        qrot = nrm.tile([128, TOKC], BF, tag="qrot")
                        for r0 in (0, HD):
                            nc.vector.tensor_copy(
                                qrot[r0:r0 + h2, :], store[r0 + h2:r0 + HD, tok0:tok0 + TOKC])
                            nc.vector.tensor_copy(
                                qrot[r0 + h2:r0 + HD, :], store[r0:r0 + h2, tok0:tok0 + TOKC])
                        cw = rope_sb[:, 2 * m, n0:n0 + TOKC]
                        sw = rope_sb[:, 2 * m + 1, n0:n0 + TOKC]
                        tms = nrm.tile([128, TOKC], BF, tag="tms")
                        nc.vector.tensor_mul(slf, slf, cw)
                        nc.vector.tensor_mul(tms[:], qrot[:], sw)
                        nc.vector.tensor_add(slf, slf, tms[:])

                    def rmul(m, tok0=tok0, st=st):
                        # multiply per-token rsqrt into the store (both heads)
                        store = qstore if m == 0 else kstore
                        rows = (0, 32) if m == 0 else (64, 96)
                        for ch in range(2):
                            rall = st["rall%d" % ch]
                            bc = psml.tile([128, 512], F32, tag="psml")
                            nc.tensor.matmul(
                                bc[0:64, :], ones_sb[rows[0]:rows[0] + 1, :],
                                rall[rows[0]:rows[0] + 1, :],
                                start=True, stop=True, tile_position=(rows[0], 0))
                            nc.tensor.matmul(
                                bc[64:128, :], ones_sb[rows[1]:rows[1] + 1, :],
                                rall[rows[1]:rows[1] + 1, :],
                                start=True, stop=True, tile_position=(rows[1], 64))
                            sl = store[:, tok0 + ch * 512:tok0 + (ch + 1) * 512]
                            nc.vector.tensor_mul(sl, sl, bc[:])

                    items += [
                        lambda f=qk_group: (f(0, 0), f(0, 1)),
                        lambda f=v_group: f(0, 0),
                        lambda f=v_group: f(0, 1),
                        lambda f=qk_group: (f(1, 0), f(1, 1)),
                        lambda f=v_group: f(1, 0),
                        lambda f=v_group: f(1, 1),
                    ]
                    lns += [squares, lambda f=rf_ln: (f(0), f(1))]
                    exps.append(lambda f=rf_exp: (f(0), f(1)))
                    tail += [
                        lambda f=rope: f(0),
                        lambda f=rmul: f(0),
                        lambda f=rope: f(1),
                        lambda f=rmul: f(1),
                    ]
                def item_lns():
                    for f in lns:
                        f()
                    if sched["last"] is not None:
                        tile.add_dep_helper(
                            actins[0].ins, sched["last"].ins,
                            sync=False, reason="act cluster head")
                    _chain(actins[0:4])
                    sched["bar"] = actins[3]

                def item_exps():
                    for f in exps:
                        f()
                    if sched["last"] is not None:
                        tile.add_dep_helper(
                            actins[4].ins, sched["last"].ins,
                            sync=False, reason="act cluster head")
                    _chain(actins[3:8])
                    sched["bar"] = actins[7]

                items.append(item_lns)
                items.append(item_exps)
                items += tail
                return items

            # ---------- attention for one batch ----------
            def attention(b, feeder):
                boff = b * N
                shard = shp.tile([QKCH, N], BF, tag="shard")
                fi = 0
                slot = 0
                prev = [None]

                def drain1(pr):
                    p0, p1, qc = pr
                    den0 = drp.tile([1, QC], F32, tag="den0")
                    den1 = drp.tile([1, QC], F32, tag="den1")
                    nc.vector.tensor_copy(den0[:], p0[64:65, :])
                    nc.vector.tensor_copy(den1[:], p1[64:65, :])
                    pvs = drp.tile([128, QC], BF, tag="pvs")
                    nc.vector.tensor_copy(pvs[0:64, :], p0[0:64, :])
                    nc.vector.tensor_copy(pvs[64:128, :], p1[0:64, :])
                    pr += [den0, den1, pvs]

                def drain2(pr):
                    den0, den1 = pr[3], pr[4]
                    drec0 = drp.tile([1, QC], F32, tag="drec0")
                    drec1 = drp.tile([1, QC], F32, tag="drec1")
                    with nc.allow_low_precision(reason="softmax denom"):
                        nc.vector.reciprocal_approx_fast(drec0[:], den0[:])
                        nc.vector.reciprocal_approx_fast(drec1[:], den1[:])
                    d0 = drp.tile([1, QC], BF, tag="d0")
                    d1 = drp.tile([1, QC], BF, tag="d1")
                    nc.vector.tensor_copy(d0[:], drec0[:])
                    nc.vector.tensor_copy(d1[:], drec1[:])
                    pr += [d0, d1]

                def drain3(pr):
                    _p0, _p1, qc, _d0f, _d1f, pvs, d0, d1 = pr
                    dbc = psml.tile([128, 512], F32, tag="psml")
                    nc.tensor.matmul(dbc[0:64, :], ones_sb[0:1, :], d0[:],
                                     start=True, stop=True, tile_position=(0, 0))
                    nc.tensor.matmul(dbc[64:128, :], ones_sb[0:1, :], d1[:],
                                     start=True, stop=True, tile_position=(0, 64))
                    nc.vector.tensor_mul(
                        shard[:, qc * QC:(qc + 1) * QC], pvs[:], dbc[:])
                    # stage this qc's two 256-token stripes for the AllToAll
                    nc.sync.dma_start(
                        a2a_in[b][2 * qc:2 * qc + 2].rearrange("j p t -> p j t"),
                        shard[:, qc * QC:(qc + 1) * QC].rearrange(
                            "p (j t) -> p j t", j=2))

                for qc in range(N // QC):
                    qoff = boff + qc * QC
                    p0 = paccp.tile([HD + 1, QC], F32, tag="pacc")
                    p1 = paccp.tile([HD + 1, QC], F32, tag="pacc")
                    last_pt = None
                    for kt in range(KT):
                        koff = boff + kt * 128
                        g = koff // 128
                        sc = scp.tile([128, 2, QC], F32, tag="sc")
                        nc.tensor.matmul(
                            sc[:, 0, :], kstore[0:HD, koff:koff + 128],
                            qstore[0:HD, qoff:qoff + QC], start=True, stop=True)
                        nc.tensor.matmul(
                            sc[:, 1, :], kstore[HD:128, koff:koff + 128],
                            qstore[HD:128, qoff:qoff + QC], start=True, stop=True)
                        if prev[0] is not None:
                            if kt == 0:
                                drain1(prev[0])
                            elif kt == 1:
                                drain2(prev[0])
                            elif kt == 2:
                                drain3(prev[0])
                                prev[0] = None
                        pt = att.tile([128, 2, QC], BF, tag="pt")
                        e = nc.scalar.activation(pt[:], sc[:], AF.Exp)
                        if sched["bar"] is not None:
                            tile.add_dep_helper(
                                e.ins, sched["bar"].ins,
                                sync=False, reason="act cluster barrier")
                            sched["bar"] = None
                        sched["last"] = e
                        if fi < len(feeder) and feeder[fi][0] <= slot:
                            feeder[fi][1]()
                            fi += 1
                        if last_pt is not None:
                            gp, ptp = last_pt
                            nc.tensor.matmul(p0[:], vstore[:, gp, 0:HD + 1],
                                             ptp[:, 0, :], start=(gp % KT == 0), stop=False)
                            nc.tensor.matmul(p1[:], vstore[:, gp, HD + 1:2 * (HD + 1)],
                                             ptp[:, 1, :], start=(gp % KT == 0), stop=False)
                        last_pt = (g, pt)
                        slot += 1
                    gp, ptp = last_pt
                    nc.tensor.matmul(p0[:], vstore[:, gp, 0:HD + 1],
                                     ptp[:, 0, :], start=False, stop=True)
                    nc.tensor.matmul(p1[:], vstore[:, gp, HD + 1:2 * (HD + 1)],
                                     ptp[:, 1, :], start=False, stop=True)
                    prev[0] = [p0, p1, qc]
                # flush remaining feeder + final drain
                while fi < len(feeder):
                    feeder[fi][1]()
                    fi += 1
                drain1(prev[0])
                drain2(prev[0])
                drain3(prev[0])
                return shard

            def reshard(b, shard):
                nc.gpsimd.collective_compute(
                    "AllToAll",
                    mybir.AluOpType.bypass,
                    replica_groups=[list(range(NCORE))],
                    ins=[a2a_in[b][:]],
                    outs=[a2a_out[b][:]],
                )

            def proj_items(b):
                st = {}

                def gather():
                    gat = gatp.tile([128, 8, ST], BF, tag="gat")
                    nc.gpsimd.dma_start(
                        gat[:], a2a_out[b][:].rearrange("c p t -> p c t"))
                    st["g"] = gat

                def mmgroup(tg, d5):
                    gat = st["g"]
                    ps = psml.tile([128, 512], F32, tag="psml")
                    for c in range(8):
                        nc.tensor.matmul(
                            ps[:], gat[:, c, tg * 128:(tg + 1) * 128],
                            wp_sb[:, c, d5 * 512:(d5 + 1) * 512],
                            start=(c == 0), stop=False)
                    nc.tensor.matmul(
                        ps[:], ones1_128[:], bp_sb[:, d5 * 512:(d5 + 1) * 512],
                        start=False, stop=True)
                    ysb = gatp.tile([128, 512], F32, tag="ysb")
                    nc.vector.tensor_copy(ysb[:], ps[:])
                    nc.gpsimd.dma_start(
                        out_d[b, tg * 128:(tg + 1) * 128, d5 * 512:(d5 + 1) * 512],
                        ysb[:])

                return [gather] + [
                    (lambda tg=tg, d5=d5: mmgroup(tg, d5))
                    for tg in range(2) for d5 in range(2)]

            # ---------- main pipeline ----------
            for it in qkv_norm_items(0):
                it()
            for b in range(B):
                feeder = []
                if b + 1 < B:
                    feeder += [(i, f) for i, f in enumerate(qkv_norm_items(b + 1))]
                if b >= 1:
                    feeder += [(28 + 2 * i, f) for i, f in enumerate(proj_items(b - 1))]
                feeder.sort(key=lambda x: x[0])
                shard = attention(b, feeder)
                reshard(b, shard)
            for f in proj_items(B - 1):
                f()

    nc.compile()
    _BUILD_CACHE["nc"] = nc
    return nc


def _host_prep(x, rope_cos, rope_sin, w_qkv, w_proj, b_proj, q_norm_w, k_norm_w):
    x = np.asarray(x, np.float32)
    xT = np.ascontiguousarray(
        x.reshape(T, C).T.reshape(8, 128, T).transpose(1, 0, 2)).astype(BF16)
    cosT = np.asarray(rope_cos, np.float32)[0, 0].T          # [hd, N]
    sinT = np.asarray(rope_sin, np.float32)[0, 0].T

    def fold(w, s):
        w = np.asarray(w, np.float32)
        cw = cosT * w[:, None] * s
        sw = np.empty_like(sinT)
        sw[:h2] = -sinT[:h2] * w[h2:HD, None] * s
        sw[h2:] = sinT[h2:] * w[0:h2, None] * s
        dup = lambda a: np.ascontiguousarray(np.concatenate([a, a], 0)).astype(BF16)
        return dup(cw), dup(sw)

    cosq, sinq = fold(q_norm_w, 1.0)     # 8 (rms) * 0.125 (softmax scale) = 1
    cosk, sink = fold(k_norm_w, 8.0)
    w_proj = np.asarray(w_proj, np.float32)
    wp = np.ascontiguousarray(
        w_proj.T.reshape(8, 128, C).transpose(1, 0, 2)).astype(BF16)
    bp = np.asarray(b_proj, np.float32).reshape(1, C).astype(BF16)
    w_qkv = np.asarray(w_qkv, np.float32)

    in_maps = []
    for r in range(NCORE):
        wq = w_qkv[QKCH * r:QKCH * (r + 1), :].T
        wk = w_qkv[C + QKCH * r:C + QKCH * (r + 1), :].T
        wv = w_qkv[2 * C + QKCH * r:2 * C + QKCH * (r + 1), :].T
        wqkvT = np.concatenate([wq, wk, wv], axis=1)         # [C, 384]
        wqkv = np.ascontiguousarray(
            wqkvT.reshape(8, 128, 3 * QKCH).transpose(1, 0, 2)).astype(BF16)
        in_maps.append({
            "xT": xT, "wqkv": wqkv, "wp": wp, "bp": bp,
            "cosq": cosq, "sinq": sinq, "cosk": cosk, "sink": sink,
        })
    return in_maps


def _run(in_maps, trace=False, **kwargs):
    nc = _build()
    return run_bass_kernel_spmd(
        nc, in_maps, core_ids=list(range(NCORE)), trace=trace, **kwargs)


def _unshard(res):
    outs = np.stack(
        [np.asarray(res.results[r]["out"], np.float32) for r in range(NCORE)])
    # outs: [core j, b, 256, C] -> y[b, j*256:(j+1)*256, :]
    return np.ascontiguousarray(outs.transpose(1, 0, 2, 3).reshape(B, N, C))


def kernel(**inputs):
    in_maps = _host_prep(**inputs)
    res = _run(in_maps)
    return _unshard(res)


# revision 27
# speedup vs baseline: 1.3380x; 1.0155x over previous
"""Trainium2 Bass kernel for nn_Attention_22539988369511 (v2).

Dense transformer attention block (B=4, N=2048, C=1024, H=16, hd=64),
sharded over 8 NeuronCores with tensor parallelism over heads (2 heads
per core), per-batch AllToAll with token striping for the output
projection.

Key design points vs v1:
 - Scores for the two heads run CONCURRENTLY in disjoint PE row-groups
   (K=64 each: rows 0-63 and 64-127), writing adjacent PSUM banks, so
   one exp activation covers both heads ([128, 1024] per k-tile).
 - All scale factors fold away: q tables carry w_q (8*0.125=1), k
   tables carry 8*w_k, and the per-token rsqrt factors are multiplied
   into qstore/kstore via K=1 broadcast matmuls, so exp has no scale AP.
 - RMS rsqrt = exp(-0.5*ln(sumsq+eps)) on ACT: ln+exp live in ONE
   activation table set -> zero table switches in the whole kernel.
   Sumsq lands at partitions {0,32,64,96} via mask matmuls so the K=1
   broadcast matmuls stay legal.
 - qkv/norm work for batch b+1 and projection for b-1 are interleaved
   into attention(b)'s PE-queue gaps (attention is ACT-bound).
 - Per-batch AllToAll (token stripes of 256) overlaps collectives with
   compute; each core owns stripe j of every batch for the projection.
"""
import os
import sys

import numpy as np
import ml_dtypes

for _p in ("/opt/trn_rl_repo", "/root/.axon_site/_ro/trn_rl_repo"):
    if os.path.isdir(_p) and _p not in sys.path:
        sys.path.append(_p)

import concourse.bass as bass
import concourse.mybir as mybir
from concourse import bacc, tile
from concourse.bass_utils import run_bass_kernel_spmd

BF16 = ml_dtypes.bfloat16
F32 = mybir.dt.float32
BF = mybir.dt.bfloat16
AF = mybir.ActivationFunctionType

NCORE = 8
B, N, C, H, HD = 4, 2048, 1024, 16, 64
HL = H // NCORE           # 2 heads per core
QKCH = HL * HD            # 128 q (or k) channels per core
T = B * N                 # 8192 tokens
ST = N // NCORE           # 256-token output stripe per core per batch
TOKC = 1024               # qkv/norm token chunk
QC = 512                  # attention q chunk
KT = N // 128             # k tiles per batch (16)
h2 = HD // 2
EPS_SUM = 64.0 * 1e-6     # eps on the 64-element sumsq

_BUILD_CACHE = {}


def _build():
    if "nc" in _BUILD_CACHE:
        return _BUILD_CACHE["nc"]
    nc = bacc.Bacc(None, target_bir_lowering=False, debug=True)

    xT_d = nc.declare_dram_parameter("xT", [128, 8, T], BF, isOutput=False)
    wqkv_d = nc.declare_dram_parameter("wqkv", [128, 8, 3 * QKCH], BF, isOutput=False)
    wp_d = nc.declare_dram_parameter("wp", [128, 8, C], BF, isOutput=False)
    bp_d = nc.declare_dram_parameter("bp", [1, C], BF, isOutput=False)
    cosq_d = nc.declare_dram_parameter("cosq", [128, N], BF, isOutput=False)
    sinq_d = nc.declare_dram_parameter("sinq", [128, N], BF, isOutput=False)
    cosk_d = nc.declare_dram_parameter("cosk", [128, N], BF, isOutput=False)
    sink_d = nc.declare_dram_parameter("sink", [128, N], BF, isOutput=False)
    out_d = nc.declare_dram_parameter("out", [B, ST, C], F32, isOutput=True)

    a2a_in = [nc.dram_tensor(f"a2a_in{b}", [NCORE, QKCH, ST], BF) for b in range(B)]
    a2a_out = [nc.dram_tensor(f"a2a_out{b}", [NCORE, QKCH, ST], BF) for b in range(B)]

    with tile.TileContext(nc) as tc:
        with (
            tc.tile_pool(name="persist", bufs=1) as pp,
            tc.tile_pool(name="xt", bufs=4) as xtp,
            tc.tile_pool(name="nrm", bufs=2) as nrm,
            tc.tile_pool(name="rfp", bufs=4) as rfp,
            tc.tile_pool(name="att", bufs=2) as att,
            tc.tile_pool(name="shp", bufs=2) as shp,
            tc.tile_pool(name="gatp", bufs=2) as gatp,
            tc.tile_pool(name="drp", bufs=2) as drp,
            tc.tile_pool(name="scp", bufs=2, space="PSUM") as scp,
            tc.tile_pool(name="paccp", bufs=2, space="PSUM") as paccp,
            tc.tile_pool(name="psml", bufs=2, space="PSUM") as psml,
        ):
            # ---- resident tiles ----
            w_sb = pp.tile([128, 8, 3 * QKCH], BF)
            wp_sb = pp.tile([128, 8, C], BF)
            bp_sb = pp.tile([1, C], BF)
            rope_sb = pp.tile([128, 4, N], BF)          # cosq|sinq|cosk|sink
            qstore = pp.tile([QKCH, T], BF)
            kstore = pp.tile([QKCH, T], BF)
            vstore = pp.tile([128, T // 128, 2 * (HD + 1)], BF)
            maskA = pp.tile([128, 97], BF)              # q sumsq -> rows 0, 32
            maskB = pp.tile([128, 97], BF)              # k sumsq -> rows 64, 96
            ones_sb = pp.tile([128, HD], BF)            # K=1 lhsT rows at any partition
            ones1_128 = pp.tile([1, 128], BF)
            eps_col = pp.tile([128, 1], F32)            # rms eps as activation bias

            nc.sync.dma_start(w_sb[:], wqkv_d[:])
            nc.sync.dma_start(wp_sb[:], wp_d[:])
            nc.sync.dma_start(bp_sb[:], bp_d[:])
            for i, td in enumerate((cosq_d, sinq_d, cosk_d, sink_d)):
                nc.sync.dma_start(rope_sb[:, i, :], td[:])
            nc.vector.memset(maskA[:], 0.0)
            nc.vector.memset(maskB[:], 0.0)
            nc.vector.memset(maskA[0:64, 0:1], 1.0)
            nc.vector.memset(maskA[64:128, 32:33], 1.0)
            nc.vector.memset(maskB[0:64, 64:65], 1.0)
            nc.vector.memset(maskB[64:128, 96:97], 1.0)
            nc.vector.memset(ones_sb[:], 1.0)
            nc.vector.memset(ones1_128[:], 1.0)
            nc.vector.memset(eps_col[:], EPS_SUM)
            nc.vector.memset(vstore[:, :, HD:HD + 1], 1.0)
            nc.vector.memset(vstore[:, :, 2 * HD + 1:2 * HD + 2], 1.0)

            # ---------- qkv + norm for one batch (two 1024-token chunks) ----
            # ACT-table discipline: all 4 Ln calls for the batch execute as
            # one consecutive cluster (one natural_log load), then the
            # exp(-0.5) calls rejoin the attention Exp stream (one exp load).
            # scheduler-order glue so Ln bursts stay contiguous between
            # attention Exps (minimizes ACT table-set reloads)
            sched = {"last": None, "bar": None}

            def _chain(instrs):
                for a, b2 in zip(instrs, instrs[1:]):
                    tile.add_dep_helper(
                        b2.ins, a.ins, sync=False, reason="act table cluster")

            def qkv_norm_items(b):
                items = []
                lns = []        # deferred matvec+ln closures, run as one item
                exps = []       # deferred exp(-0.5) closures
                tail = []       # rope + r-multiply items
                actins = []     # ACT instructions to keep contiguous
                for ti in (2 * b, 2 * b + 1):
                    tok0 = ti * TOKC
                    n0 = tok0 % N
                    st = {}

                    def xload(tok0=tok0, st=st):
                        for t5 in range(2):
                            tk0 = tok0 + t5 * 512
                            xt = xtp.tile([128, 8, 512], BF, tag="xt")
                            nc.sync.dma_start(xt[:], xT_d[:, :, tk0:tk0 + 512])
                            st[t5] = xt

                    def qk_group(t5, m, ti=ti, tok0=tok0, st=st):
                        tk0 = tok0 + t5 * 512
                        xt = st[t5]
                        store = qstore if m == 0 else kstore
                        ps = psml.tile([128, 512], F32, tag="psml")
                        for c in range(8):
                            nc.tensor.matmul(
                                ps[:], w_sb[:, c, m * QKCH:(m + 1) * QKCH],
                                xt[:, c, :], start=(c == 0), stop=(c == 7))
                        nc.vector.tensor_copy(store[:, tk0:tk0 + 512], ps[:])

                    def v_group(t5, t1pair, tok0=tok0, st=st):
                        tk0 = tok0 + t5 * 512
                        xt = st[t5]
                        for t1 in (2 * t1pair, 2 * t1pair + 1):
                            ps = psml.tile([128, 512], F32, tag="psml")
                            for c in range(8):
                                nc.tensor.matmul(
                                    ps[:, 0:128],
                                    xt[:, c, t1 * 128:(t1 + 1) * 128],
                                    w_sb[:, c, 2 * QKCH:3 * QKCH],
                                    start=(c == 0), stop=(c == 7))
                            g = (tk0 // 128) + t1
                            nc.vector.tensor_copy(
                                vstore[:, g, :].rearrange(
                                    "p (a b) -> p a b", b=HD + 1)[:, :, 0:HD],
                                ps[:, 0:128].rearrange("p (a b) -> p a b", b=HD))

                    def squares(tok0=tok0, st=st):
                        sqq = nrm.tile([128, TOKC], BF, tag="sqq")
                        sqk = nrm.tile([128, TOKC], BF, tag="sqk")
                        nc.vector.tensor_mul(
                            sqq[:], qstore[:, tok0:tok0 + TOKC], qstore[:, tok0:tok0 + TOKC])
                        nc.vector.tensor_mul(
                            sqk[:], kstore[:, tok0:tok0 + TOKC], kstore[:, tok0:tok0 + TOKC])
                        st["sqq"], st["sqk"] = sqq, sqk

                    def rf_ln(ch, st=st):
                        ps = psml.tile([128, 512], F32, tag="psml")
                        nc.tensor.matmul(
                            ps[0:97, :], maskA[:],
                            st["sqq"][:, ch * 512:(ch + 1) * 512],
                            start=True, stop=False)
                        nc.tensor.matmul(
                            ps[0:97, :], maskB[:],
                            st["sqk"][:, ch * 512:(ch + 1) * 512],
                            start=False, stop=True)
                        lnscr = rfp.tile([97, 512], F32, tag="lnscr")
                        actins.append(nc.scalar.activation(
                            lnscr[:], ps[0:97, :], AF.Ln, bias=eps_col[0:97, :]))
                        st["ln%d" % ch] = lnscr

                    def rf_exp(ch, st=st):
                        rall = rfp.tile([97, 512], BF, tag="rall")
                        actins.append(nc.scalar.activation(
                            rall[:], st["ln%d" % ch], AF.Exp, scale=-0.5))
                        st["rall%d" % ch] = rall

                    def rope(m, tok0=tok0, n0=n0):
                        store = qstore if m == 0 else kstore
                        slf = store[:, tok0:tok0 + TOKC]
                        qrot = nrm.tile([128, TOKC], BF, tag="qrot")
                        for r0 in (0, HD):
                            nc.vector.tensor_copy(
                                qrot[r0:r0 + h2, :], store[r0 + h2:r0 + HD, tok0:tok0 + TOKC])
                            nc.vector.tensor_copy(
                                qrot[r0 + h2:r0 + HD, :], store[r0:r0 + h2, tok0:tok0 + TOKC])
                        cw = rope_sb[:, 2 * m, n0:n0 + TOKC]
                        sw = rope_sb[:, 2 * m + 1, n0:n0 + TOKC]
                        tms = nrm.tile([128, TOKC], BF, tag="tms")
                        nc.vector.tensor_mul(slf, slf, cw)
                        nc.vector.tensor_mul(tms[:], qrot[:], sw)
                        nc.vector.tensor_add(slf, slf, tms[:])

                    def rmul(m, tok0=tok0, st=st):
                        # multiply per-token rsqrt into the store (both heads)
                        store = qstore if m == 0 else kstore
                        rows = (0, 32) if m == 0 else (64, 96)
                        for ch in range(2):
                            rall = st["rall%d" % ch]
                            bc = psml.tile([128, 512], F32, tag="psml")
                            nc.tensor.matmul(
                                bc[0:64, :], ones_sb[rows[0]:rows[0] + 1, :],
                                rall[rows[0]:rows[0] + 1, :],
                                start=True, stop=True, tile_position=(rows[0], 0))
                            nc.tensor.matmul(
                                bc[64:128, :], ones_sb[rows[1]:rows[1] + 1, :],
                                rall[rows[1]:rows[1] + 1, :],
                                start=True, stop=True, tile_position=(rows[1], 64))
                            sl = store[:, tok0 + ch * 512:tok0 + (ch + 1) * 512]
                            nc.vector.tensor_mul(sl, sl, bc[:])

                    items.append((xload, squares, qk_group, v_group))
                    lns.append(lambda f=rf_ln: (f(0), f(1)))
                    exps.append(lambda f=rf_exp: (f(0), f(1)))
                    tail += [
                        lambda f=rope: f(0),
                        lambda f=rmul: f(0),
                        lambda f=rope: f(1),
                        lambda f=rmul: f(1),
                    ]
                def item_lns():
                    for f in lns:
                        f()
                    if sched["last"] is not None:
                        tile.add_dep_helper(
                            actins[0].ins, sched["last"].ins,
                            sync=False, reason="act cluster head")
                    _chain(actins[0:4])
                    sched["bar"] = actins[3]

                def item_exps():
                    for f in exps:
                        f()
                    if sched["last"] is not None:
                        tile.add_dep_helper(
                            actins[4].ins, sched["last"].ins,
                            sync=False, reason="act cluster head")
                    _chain(actins[3:8])
                    sched["bar"] = actins[7]

                (xl0, sq0, qk0, v0), (xl1, sq1, qk1, v1) = items
                # (slot, closure): DMA-only prefetch first so MM groups never
                # head-of-line block the PE queue on an HBM load
                slotted = [
                    (0, xl0), (1, xl1),
                    (3, lambda: (qk0(0, 0), qk0(0, 1))),
                    (5, lambda: (qk0(1, 0), qk0(1, 1))),
                    (7, lambda: (qk1(0, 0), qk1(0, 1))),
                    (9, lambda: (qk1(1, 0), qk1(1, 1))),
                    (10, sq0),
                    (11, lambda: v0(0, 0)), (12, lambda: v0(0, 1)),
                    (13, sq1),
                    (14, lambda: v0(1, 0)), (15, lambda: v0(1, 1)),
                    (16, lambda: v1(0, 0)), (17, lambda: v1(0, 1)),
                    (18, lambda: v1(1, 0)), (19, lambda: v1(1, 1)),
                    (21, item_lns),
                    (23, item_exps),
                ]
                slotted += [(25 + 2 * i, f) for i, f in enumerate(tail)]
                return slotted

            # ---------- attention for one batch ----------
            def attention(b, feeder):
                boff = b * N
                shard = shp.tile([QKCH, N], BF, tag="shard")
                fi = 0
                slot = 0
                prev = [None]

                def drain1(pr):
                    p0, p1, qc = pr
                    den0 = drp.tile([1, QC], F32, tag="den0")
                    den1 = drp.tile([1, QC], F32, tag="den1")
                    nc.vector.tensor_copy(den0[:], p0[64:65, :])
                    nc.vector.tensor_copy(den1[:], p1[64:65, :])
                    pvs = drp.tile([128, QC], BF, tag="pvs")
                    nc.vector.tensor_copy(pvs[0:64, :], p0[0:64, :])
                    nc.vector.tensor_copy(pvs[64:128, :], p1[0:64, :])
                    pr += [den0, den1, pvs]

                def drain2(pr):
                    den0, den1 = pr[3], pr[4]
                    drec0 = drp.tile([1, QC], F32, tag="drec0")
                    drec1 = drp.tile([1, QC], F32, tag="drec1")
                    with nc.allow_low_precision(reason="softmax denom"):
                        nc.vector.reciprocal_approx_fast(drec0[:], den0[:])
                        nc.vector.reciprocal_approx_fast(drec1[:], den1[:])
                    d0 = drp.tile([1, QC], BF, tag="d0")
                    d1 = drp.tile([1, QC], BF, tag="d1")
                    nc.vector.tensor_copy(d0[:], drec0[:])
                    nc.vector.tensor_copy(d1[:], drec1[:])
                    pr += [d0, d1]

                def drain3(pr):
                    _p0, _p1, qc, _d0f, _d1f, pvs, d0, d1 = pr
                    dbc = psml.tile([128, 512], F32, tag="psml")
                    nc.tensor.matmul(dbc[0:64, :], ones_sb[0:1, :], d0[:],
                                     start=True, stop=True, tile_position=(0, 0))
                    nc.tensor.matmul(dbc[64:128, :], ones_sb[0:1, :], d1[:],
                                     start=True, stop=True, tile_position=(0, 64))
                    nc.vector.tensor_mul(
                        shard[:, qc * QC:(qc + 1) * QC], pvs[:], dbc[:])
                    # stage this qc's two 256-token stripes for the AllToAll
                    nc.sync.dma_start(
                        a2a_in[b][2 * qc:2 * qc + 2].rearrange("j p t -> p j t"),
                        shard[:, qc * QC:(qc + 1) * QC].rearrange(
                            "p (j t) -> p j t", j=2))

                for qc in range(N // QC):
                    qoff = boff + qc * QC
                    p0 = paccp.tile([HD + 1, QC], F32, tag="pacc")
                    p1 = paccp.tile([HD + 1, QC], F32, tag="pacc")
                    last_pt = None
                    for kt in range(KT):
                        koff = boff + kt * 128
                        g = koff // 128
                        sc = scp.tile([128, 2, QC], F32, tag="sc")
                        nc.tensor.matmul(
                            sc[:, 0, :], kstore[0:HD, koff:koff + 128],
                            qstore[0:HD, qoff:qoff + QC], start=True, stop=True)
                        nc.tensor.matmul(
                            sc[:, 1, :], kstore[HD:128, koff:koff + 128],
                            qstore[HD:128, qoff:qoff + QC], start=True, stop=True)
                        if prev[0] is not None:
                            if kt == 0:
                                drain1(prev[0])
                            elif kt == 1:
                                drain2(prev[0])
                            elif kt == 2:
                                drain3(prev[0])
                                prev[0] = None
                        pt = att.tile([128, 2, QC], BF, tag="pt")
                        e = nc.scalar.activation(pt[:], sc[:], AF.Exp)
                        if sched["bar"] is not None:
                            tile.add_dep_helper(
                                e.ins, sched["bar"].ins,
                                sync=False, reason="act cluster barrier")
                            sched["bar"] = None
                        sched["last"] = e
                        if fi < len(feeder) and feeder[fi][0] <= slot:
                            feeder[fi][1]()
                            fi += 1
                        if last_pt is not None:
                            gp, ptp = last_pt
                            nc.tensor.matmul(p0[:], vstore[:, gp, 0:HD + 1],
                                             ptp[:, 0, :], start=(gp % KT == 0), stop=False)
                            nc.tensor.matmul(p1[:], vstore[:, gp, HD + 1:2 * (HD + 1)],
                                             ptp[:, 1, :], start=(gp % KT == 0), stop=False)
                        last_pt = (g, pt)
                        slot += 1
                    gp, ptp = last_pt
                    nc.tensor.matmul(p0[:], vstore[:, gp, 0:HD + 1],
                                     ptp[:, 0, :], start=False, stop=True)
                    nc.tensor.matmul(p1[:], vstore[:, gp, HD + 1:2 * (HD + 1)],
                                     ptp[:, 1, :], start=False, stop=True)
                    prev[0] = [p0, p1, qc]
                # flush remaining feeder + final drain
                while fi < len(feeder):
                    feeder[fi][1]()
                    fi += 1
                drain1(prev[0])
                drain2(prev[0])
                drain3(prev[0])
                return shard

            def reshard(b, shard):
                nc.gpsimd.collective_compute(
                    "AllToAll",
                    mybir.AluOpType.bypass,
                    replica_groups=[list(range(NCORE))],
                    ins=[a2a_in[b][:]],
                    outs=[a2a_out[b][:]],
                )

            def proj_items(b):
                st = {}

                def gather():
                    gat = gatp.tile([128, 8, ST], BF, tag="gat")
                    nc.gpsimd.dma_start(
                        gat[:], a2a_out[b][:].rearrange("c p t -> p c t"))
                    st["g"] = gat

                def mmgroup(tg, d5):
                    gat = st["g"]
                    ps = psml.tile([128, 512], F32, tag="psml")
                    for c in range(8):
                        nc.tensor.matmul(
                            ps[:], gat[:, c, tg * 128:(tg + 1) * 128],
                            wp_sb[:, c, d5 * 512:(d5 + 1) * 512],
                            start=(c == 0), stop=False)
                    nc.tensor.matmul(
                        ps[:], ones1_128[:], bp_sb[:, d5 * 512:(d5 + 1) * 512],
                        start=False, stop=True)
                    ysb = gatp.tile([128, 512], F32, tag="ysb")
                    nc.vector.tensor_copy(ysb[:], ps[:])
                    nc.gpsimd.dma_start(
                        out_d[b, tg * 128:(tg + 1) * 128, d5 * 512:(d5 + 1) * 512],
                        ysb[:])

                return [gather] + [
                    (lambda tg=tg, d5=d5: mmgroup(tg, d5))
                    for tg in range(2) for d5 in range(2)]

            # ---------- main pipeline ----------
            for _s, it in sorted(qkv_norm_items(0), key=lambda x: x[0]):
                it()
            for b in range(B):
                feeder = []
                if b + 1 < B:
                    feeder += qkv_norm_items(b + 1)
                if b >= 1:
                    feeder += [(41 + 2 * i, f) for i, f in enumerate(proj_items(b - 1))]
                feeder.sort(key=lambda x: x[0])
                shard = attention(b, feeder)
                reshard(b, shard)
            for f in proj_items(B - 1):
                f()

    nc.compile()
    _BUILD_CACHE["nc"] = nc
    return nc


def _host_prep(x, rope_cos, rope_sin, w_qkv, w_proj, b_proj, q_norm_w, k_norm_w):
    x = np.asarray(x, np.float32)
    xT = np.ascontiguousarray(
        x.reshape(T, C).T.reshape(8, 128, T).transpose(1, 0, 2)).astype(BF16)
    cosT = np.asarray(rope_cos, np.float32)[0, 0].T          # [hd, N]
    sinT = np.asarray(rope_sin, np.float32)[0, 0].T

    def fold(w, s):
        w = np.asarray(w, np.float32)
        cw = cosT * w[:, None] * s
        sw = np.empty_like(sinT)
        sw[:h2] = -sinT[:h2] * w[h2:HD, None] * s
        sw[h2:] = sinT[h2:] * w[0:h2, None] * s
        dup = lambda a: np.ascontiguousarray(np.concatenate([a, a], 0)).astype(BF16)
        return dup(cw), dup(sw)

    cosq, sinq = fold(q_norm_w, 1.0)     # 8 (rms) * 0.125 (softmax scale) = 1
    cosk, sink = fold(k_norm_w, 8.0)
    w_proj = np.asarray(w_proj, np.float32)
    wp = np.ascontiguousarray(
        w_proj.T.reshape(8, 128, C).transpose(1, 0, 2)).astype(BF16)
    bp = np.asarray(b_proj, np.float32).reshape(1, C).astype(BF16)
    w_qkv = np.asarray(w_qkv, np.float32)

    in_maps = []
    for r in range(NCORE):
        wq = w_qkv[QKCH * r:QKCH * (r + 1), :].T
        wk = w_qkv[C + QKCH * r:C + QKCH * (r + 1), :].T
        wv = w_qkv[2 * C + QKCH * r:2 * C + QKCH * (r + 1), :].T
        wqkvT = np.concatenate([wq, wk, wv], axis=1)         # [C, 384]
        wqkv = np.ascontiguousarray(
            wqkvT.reshape(8, 128, 3 * QKCH).transpose(1, 0, 2)).astype(BF16)
        in_maps.append({
            "xT": xT, "wqkv": wqkv, "wp": wp, "bp": bp,
            "cosq": cosq, "sinq": sinq, "cosk": cosk, "sink": sink,
        })
    return in_maps


def _run(in_maps, trace=False, **kwargs):
    nc = _build()
    return run_bass_kernel_spmd(
        nc, in_maps, core_ids=list(range(NCORE)), trace=trace, **kwargs)


def _unshard(res):
    outs = np.stack(
        [np.asarray(res.results[r]["out"], np.float32) for r in range(NCORE)])
    # outs: [core j, b, 256, C] -> y[b, j*256:(j+1)*256, :]
    return np.ascontiguousarray(outs.transpose(1, 0, 2, 3).reshape(B, N, C))


def kernel(**inputs):
    in_maps = _host_prep(**inputs)
    res = _run(in_maps)
    return _unshard(res)


# revision 41
# speedup vs baseline: 1.3772x; 1.0293x over previous
"""Trainium2 Bass kernel for nn_Attention_22539988369511 (v2).

Dense transformer attention block (B=4, N=2048, C=1024, H=16, hd=64),
sharded over 8 NeuronCores with tensor parallelism over heads (2 heads
per core), per-batch AllToAll with token striping for the output
projection.

Key design points vs v1:
 - Scores for the two heads run CONCURRENTLY in disjoint PE row-groups
   (K=64 each: rows 0-63 and 64-127), writing adjacent PSUM banks, so
   one exp activation covers both heads ([128, 1024] per k-tile).
 - All scale factors fold away: q tables carry w_q (8*0.125=1), k
   tables carry 8*w_k, and the per-token rsqrt factors are multiplied
   into qstore/kstore via K=1 broadcast matmuls, so exp has no scale AP.
 - RMS rsqrt = exp(-0.5*ln(sumsq+eps)) on ACT: ln+exp live in ONE
   activation table set -> zero table switches in the whole kernel.
   Sumsq lands at partitions {0,32,64,96} via mask matmuls so the K=1
   broadcast matmuls stay legal.
 - qkv/norm work for batch b+1 and projection for b-1 are interleaved
   into attention(b)'s PE-queue gaps (attention is ACT-bound).
 - Per-batch AllToAll (token stripes of 256) overlaps collectives with
   compute; each core owns stripe j of every batch for the projection.
"""
import os
import sys

import numpy as np
import ml_dtypes

for _p in ("/opt/trn_rl_repo", "/root/.axon_site/_ro/trn_rl_repo"):
    if os.path.isdir(_p) and _p not in sys.path:
        sys.path.append(_p)

import concourse.bass as bass
import concourse.mybir as mybir
from concourse import bacc, tile
from concourse.bass_utils import run_bass_kernel_spmd

BF16 = ml_dtypes.bfloat16
F32 = mybir.dt.float32
BF = mybir.dt.bfloat16
AF = mybir.ActivationFunctionType

NCORE = 8
B, N, C, H, HD = 4, 2048, 1024, 16, 64
HL = H // NCORE           # 2 heads per core
QKCH = HL * HD            # 128 q (or k) channels per core
T = B * N                 # 8192 tokens
ST = N // NCORE           # 256-token output stripe per core per batch
TOKC = 1024               # qkv/norm token chunk
QC = 512                  # attention q chunk
KT = N // 128             # k tiles per batch (16)
h2 = HD // 2
EPS_SUM = 64.0 * 1e-6     # eps on the 64-element sumsq

_BUILD_CACHE = {}


def _build():
    if "nc" in _BUILD_CACHE:
        return _BUILD_CACHE["nc"]
    nc = bacc.Bacc(None, target_bir_lowering=False, debug=True)

    xT_d = nc.declare_dram_parameter("xT", [128, 8, T], BF, isOutput=False)
    wqkv_d = nc.declare_dram_parameter("wqkv", [128, 8, 3 * QKCH], BF, isOutput=False)
    wp_d = nc.declare_dram_parameter("wp", [128, 8, C], BF, isOutput=False)
    bp_d = nc.declare_dram_parameter("bp", [1, C], BF, isOutput=False)
    cosq_d = nc.declare_dram_parameter("cosq", [128, N], BF, isOutput=False)
    sinq_d = nc.declare_dram_parameter("sinq", [128, N], BF, isOutput=False)
    cosk_d = nc.declare_dram_parameter("cosk", [128, N], BF, isOutput=False)
    sink_d = nc.declare_dram_parameter("sink", [128, N], BF, isOutput=False)
    eye_d = nc.declare_dram_parameter("eye", [128, 128], BF, isOutput=False)
    out_d = nc.declare_dram_parameter("out", [B, ST, C], F32, isOutput=True)

    a2a_in = [nc.dram_tensor(f"a2a_in{b}", [NCORE, QKCH, ST], BF) for b in range(B)]
    a2a_out = [nc.dram_tensor(f"a2a_out{b}", [NCORE, QKCH, ST], BF) for b in range(B)]

    with tile.TileContext(nc) as tc:
        with (
            tc.tile_pool(name="persist", bufs=1) as pp,
            tc.tile_pool(name="xt", bufs=4) as xtp,
            tc.tile_pool(name="nrm", bufs=2) as nrm,
            tc.tile_pool(name="rfp", bufs=4) as rfp,
            tc.tile_pool(name="att", bufs=3) as att,
            tc.tile_pool(name="shp", bufs=2) as shp,
            tc.tile_pool(name="gatp", bufs=2) as gatp,
            tc.tile_pool(name="drp", bufs=2) as drp,
            tc.tile_pool(name="scp", bufs=2, space="PSUM") as scp,
            tc.tile_pool(name="paccp", bufs=2, space="PSUM") as paccp,
            tc.tile_pool(name="psml", bufs=2, space="PSUM") as psml,
        ):
            # ---- resident tiles ----
            w_sb = pp.tile([128, 8, 3 * QKCH], BF)
            wp_sb = pp.tile([128, 8, C], BF)
            bp_sb = pp.tile([1, C], BF)
            rope_sb = pp.tile([128, 4, N], BF)          # cosq|sinq|cosk|sink
            qstore = pp.tile([QKCH, T], BF)
            kstore = pp.tile([QKCH, T], BF)
            # per head: [V (64) | ones (32)] so P@V emits the softmax
            # denominator pre-broadcast across 32 partitions
            VW = HD + 32
            vstore = pp.tile([128, T // 128, 2 * VW], BF)
            eye_sb = pp.tile([128, 128], BF)
            maskA = pp.tile([128, 97], BF)              # q sumsq -> rows 0, 32
            maskB = pp.tile([128, 97], BF)              # k sumsq -> rows 64, 96
            ones_sb = pp.tile([128, HD], BF)            # K=1 lhsT rows at any partition
            ones1_128 = pp.tile([1, 128], BF)
            eps_col = pp.tile([128, 1], F32)            # rms eps as activation bias

            nc.sync.dma_start(w_sb[:], wqkv_d[:])
            nc.sync.dma_start(wp_sb[:], wp_d[:])
            nc.sync.dma_start(bp_sb[:], bp_d[:])
            nc.sync.dma_start(eye_sb[:], eye_d[:])
            for i, td in enumerate((cosq_d, sinq_d, cosk_d, sink_d)):
                nc.sync.dma_start(rope_sb[:, i, :], td[:])
            nc.vector.memset(maskA[:], 0.0)
            nc.vector.memset(maskB[:], 0.0)
            nc.vector.memset(maskA[0:64, 0:1], 1.0)
            nc.vector.memset(maskA[64:128, 32:33], 1.0)
            nc.vector.memset(maskB[0:64, 64:65], 1.0)
            nc.vector.memset(maskB[64:128, 96:97], 1.0)
            nc.vector.memset(ones_sb[:], 1.0)
            nc.vector.memset(ones1_128[:], 1.0)
            nc.vector.memset(eps_col[:], EPS_SUM)
            nc.vector.memset(vstore[:, :, HD:VW], 1.0)
            nc.vector.memset(vstore[:, :, VW + HD:2 * VW], 1.0)

            # ---------- qkv + norm for one batch (two 1024-token chunks) ----
            # ACT-table discipline: all 4 Ln calls for the batch execute as
            # one consecutive cluster (one natural_log load), then the
            # exp(-0.5) calls rejoin the attention Exp stream (one exp load).
            # scheduler-order glue so Ln bursts stay contiguous between
            # attention Exps (minimizes ACT table-set reloads)
            sched = {"last": None, "bar": None}

            def _chain(instrs):
                for a, b2 in zip(instrs, instrs[1:]):
                    tile.add_dep_helper(
                        b2.ins, a.ins, sync=False, reason="act table cluster")

            def qkv_norm_items(b):
                items = []
                lns = []        # deferred matvec+ln closures, run as one item
                exps = []       # deferred exp(-0.5) closures
                tail = []       # rope + r-multiply items
                actins = []     # ACT instructions to keep contiguous
                for ti in (2 * b, 2 * b + 1):
                    tok0 = ti * TOKC
                    n0 = tok0 % N
                    st = {}

                    def xload(tok0=tok0, st=st):
                        for t5 in range(2):
                            tk0 = tok0 + t5 * 512
                            xt = xtp.tile([128, 8, 512], BF, tag="xt")
                            nc.sync.dma_start(xt[:], xT_d[:, :, tk0:tk0 + 512])
                            st[t5] = xt

                    def qk_group(t5, m, ti=ti, tok0=tok0, st=st):
                        tk0 = tok0 + t5 * 512
                        xt = st[t5]
                        store = qstore if m == 0 else kstore
                        ps = psml.tile([128, 512], F32, tag="psml")
                        for c in range(8):
                            nc.tensor.matmul(
                                ps[:], w_sb[:, c, m * QKCH:(m + 1) * QKCH],
                                xt[:, c, :], start=(c == 0), stop=(c == 7))
                        nc.vector.tensor_copy(store[:, tk0:tk0 + 512], ps[:])

                    def v_mm(t5, tok0=tok0, st=st):
                        # v as [vch, tok] (weights stationary), then cast to
                        # SBUF for the PE transposes
                        xt = st[t5]
                        ps = psml.tile([128, 512], F32, tag="psml")
                        for c in range(8):
                            nc.tensor.matmul(
                                ps[:], w_sb[:, c, 2 * QKCH:3 * QKCH],
                                xt[:, c, :], start=(c == 0), stop=(c == 7))
                        vtmp = nrm.tile([128, 512], BF, tag="vtmp")
                        nc.vector.tensor_copy(vtmp[:], ps[:])
                        st["v%d" % t5] = vtmp

                    def v_tr(t5, tok0=tok0, st=st):
                        tk0 = tok0 + t5 * 512
                        vtmp = st["v%d" % t5]
                        ps = psml.tile([128, 512], F32, tag="psml")
                        pt4 = ps[:].bitcast(BF)
                        for t1 in range(4):
                            nc.tensor.transpose(
                                pt4[:, t1 * 128:(t1 + 1) * 128],
                                vtmp[:, t1 * 128:(t1 + 1) * 128], eye_sb[:])
                        for t1 in range(4):
                            g = (tk0 // 128) + t1
                            nc.vector.tensor_copy(
                                vstore[:, g, :].rearrange(
                                    "p (a b) -> p a b", b=VW)[:, :, 0:HD],
                                pt4[:, t1 * 128:(t1 + 1) * 128].rearrange(
                                    "p (a b) -> p a b", b=HD))

                    def squares(tok0=tok0, st=st):
                        sqq = nrm.tile([128, TOKC], BF, tag="sqq")
                        sqk = nrm.tile([128, TOKC], BF, tag="sqk")
                        nc.vector.tensor_mul(
                            sqq[:], qstore[:, tok0:tok0 + TOKC], qstore[:, tok0:tok0 + TOKC])
                        nc.vector.tensor_mul(
                            sqk[:], kstore[:, tok0:tok0 + TOKC], kstore[:, tok0:tok0 + TOKC])
                        st["sqq"], st["sqk"] = sqq, sqk

                    def rf_ln(ch, st=st):
                        ps = psml.tile([128, 512], F32, tag="psml")
                        nc.tensor.matmul(
                            ps[0:97, :], maskA[:],
                            st["sqq"][:, ch * 512:(ch + 1) * 512],
                            start=True, stop=False)
                        nc.tensor.matmul(
                            ps[0:97, :], maskB[:],
                            st["sqk"][:, ch * 512:(ch + 1) * 512],
                            start=False, stop=True)
                        lnscr = rfp.tile([97, 512], F32, tag="lnscr")
                        actins.append(nc.scalar.activation(
                            lnscr[:], ps[0:97, :], AF.Ln, bias=eps_col[0:97, :]))
                        st["ln%d" % ch] = lnscr

                    def rf_exp(ch, st=st):
                        rall = rfp.tile([97, 512], BF, tag="rall")
                        actins.append(nc.scalar.activation(
                            rall[:], st["ln%d" % ch], AF.Exp, scale=-0.5))
                        st["rall%d" % ch] = rall

                    def rope(m, tok0=tok0, n0=n0):
                        store = qstore if m == 0 else kstore
                        slf = store[:, tok0:tok0 + TOKC]
                        qrot = nrm.tile([128, TOKC], BF, tag="qrot")
                        for r0 in (0, HD):
                            nc.vector.tensor_copy(
                                qrot[r0:r0 + h2, :], store[r0 + h2:r0 + HD, tok0:tok0 + TOKC])
                            nc.vector.tensor_copy(
                                qrot[r0 + h2:r0 + HD, :], store[r0:r0 + h2, tok0:tok0 + TOKC])
                        cw = rope_sb[:, 2 * m, n0:n0 + TOKC]
                        sw = rope_sb[:, 2 * m + 1, n0:n0 + TOKC]
                        tms = nrm.tile([128, TOKC], BF, tag="tms")
                        nc.vector.tensor_mul(slf, slf, cw)
                        nc.vector.tensor_mul(tms[:], qrot[:], sw)
                        nc.vector.tensor_add(slf, slf, tms[:])

                    def rmul(m, tok0=tok0, st=st):
                        # multiply per-token rsqrt into the store (both heads)
                        store = qstore if m == 0 else kstore
                        rows = (0, 32) if m == 0 else (64, 96)
                        for ch in range(2):
                            rall = st["rall%d" % ch]
                            bc = psml.tile([128, 512], F32, tag="psml")
                            nc.tensor.matmul(
                                bc[0:64, :], ones_sb[rows[0]:rows[0] + 1, :],
                                rall[rows[0]:rows[0] + 1, :],
                                start=True, stop=True, tile_position=(rows[0], 0))
                            nc.tensor.matmul(
                                bc[64:128, :], ones_sb[rows[1]:rows[1] + 1, :],
                                rall[rows[1]:rows[1] + 1, :],
                                start=True, stop=True, tile_position=(rows[1], 64))
                            sl = store[:, tok0 + ch * 512:tok0 + (ch + 1) * 512]
                            nc.vector.tensor_mul(sl, sl, bc[:])

                    items.append((xload, squares, qk_group, v_mm, v_tr))
                    lns.append(lambda f=rf_ln: (f(0), f(1)))
                    exps.append(lambda f=rf_exp: (f(0), f(1)))
                    tail += [
                        lambda f=rope: f(0),
                        lambda f=rmul: f(0),
                        lambda f=rope: f(1),
                        lambda f=rmul: f(1),
                    ]
                def item_lns():
                    for f in lns:
                        f()
                    if sched["last"] is not None:
                        tile.add_dep_helper(
                            actins[0].ins, sched["last"].ins,
                            sync=False, reason="act cluster head")
                    _chain(actins[0:4])
                    sched["bar"] = actins[3]

                def item_exps():
                    for f in exps:
                        f()
                    if sched["last"] is not None:
                        tile.add_dep_helper(
                            actins[4].ins, sched["last"].ins,
                            sync=False, reason="act cluster head")
                    _chain(actins[3:8])
                    sched["bar"] = actins[7]

                (xl0, sq0, qk0, vm0, vt0), (xl1, sq1, qk1, vm1, vt1) = items
                # (slot, closure): DMA-only prefetch first so MM groups never
                # head-of-line block the PE queue on an HBM load; one short
                # PE burst per slot so the exp stream never starves
                slotted = [
                    (0, xl0), (1, xl1),
                    (2, lambda: qk0(0, 0)), (3, lambda: qk0(0, 1)),
                    (4, lambda: qk0(1, 0)), (5, lambda: qk0(1, 1)),
                    (6, lambda: qk1(0, 0)), (7, lambda: qk1(0, 1)),
                    (8, lambda: qk1(1, 0)), (9, lambda: qk1(1, 1)),
                    (10, sq0),
                    (11, lambda: vm0(0)), (12, lambda: vt0(0)),
                    (13, lambda: vm0(1)), (14, lambda: vt0(1)),
                    (15, sq1),
                    (16, lambda: vm1(0)), (17, lambda: vt1(0)),
                    (18, lambda: vm1(1)), (19, lambda: vt1(1)),
                    (21, item_lns),
                    (23, item_exps),
                ]
                slotted += [(25 + 2 * i, f) for i, f in enumerate(tail)]
                return slotted

            # ---------- attention for one batch ----------
            def attention(b, feeder):
                boff = b * N
                shard = shp.tile([QKCH, N], BF, tag="shard")
                fi = 0
                slot = 0
                prev = [None]

                def drain1(pr):
                    p0, p1, qc = pr
                    den0 = drp.tile([32, QC], F32, tag="den0")
                    den1 = drp.tile([32, QC], F32, tag="den1")
                    nc.vector.tensor_copy(den0[:], p0[64:96, :])
                    nc.vector.tensor_copy(den1[:], p1[64:96, :])
                    pvs = drp.tile([128, QC], BF, tag="pvs")
                    nc.vector.tensor_copy(pvs[0:64, :], p0[0:64, :])
                    nc.vector.tensor_copy(pvs[64:128, :], p1[0:64, :])
                    pr += [den0, den1, pvs]

                def drain2(pr):
                    den0, den1 = pr[3], pr[4]
                    drec0 = drp.tile([32, QC], F32, tag="drec0")
                    drec1 = drp.tile([32, QC], F32, tag="drec1")
                    with nc.allow_low_precision(reason="softmax denom"):
                        nc.vector.reciprocal_approx_fast(drec0[:], den0[:])
                        nc.vector.reciprocal_approx_fast(drec1[:], den1[:])
                    d01 = drp.tile([128, QC], BF, tag="d01")
                    nc.vector.tensor_copy(d01[0:32, :], drec0[:])
                    nc.vector.tensor_copy(d01[32:64, :], drec0[:])
                    nc.vector.tensor_copy(d01[64:96, :], drec1[:])
                    nc.vector.tensor_copy(d01[96:128, :], drec1[:])
                    pr.append(d01)

                def drain3(pr):
                    _p0, _p1, qc, _d0f, _d1f, pvs, d01 = pr
                    nc.vector.tensor_mul(
                        shard[:, qc * QC:(qc + 1) * QC], pvs[:], d01[:])
                    # stage this qc's two 256-token stripes for the AllToAll
                    nc.sync.dma_start(
                        a2a_in[b][2 * qc:2 * qc + 2].rearrange("j p t -> p j t"),
                        shard[:, qc * QC:(qc + 1) * QC].rearrange(
                            "p (j t) -> p j t", j=2))

                for qc in range(N // QC):
                    qoff = boff + qc * QC
                    p0 = paccp.tile([96, QC], F32, tag="pacc")
                    p1 = paccp.tile([96, QC], F32, tag="pacc")
                    pend = []      # pts awaiting PV, two k-tiles behind exp

                    def pv_flush(last):
                        while pend and (len(pend) > 2 or last):
                            gp, ptp = pend.pop(0)
                            nc.tensor.matmul(
                                p0[:], vstore[:, gp, 0:VW], ptp[:, 0, :],
                                start=(gp % KT == 0), stop=(last and not pend))
                            nc.tensor.matmul(
                                p1[:], vstore[:, gp, VW:2 * VW], ptp[:, 1, :],
                                start=(gp % KT == 0), stop=(last and not pend))

                    for kt in range(KT):
                        koff = boff + kt * 128
                        g = koff // 128
                        sc = scp.tile([128, 2, QC], F32, tag="sc")
                        nc.tensor.matmul(
                            sc[:, 0, :], kstore[0:HD, koff:koff + 128],
                            qstore[0:HD, qoff:qoff + QC], start=True, stop=True)
                        nc.tensor.matmul(
                            sc[:, 1, :], kstore[HD:128, koff:koff + 128],
                            qstore[HD:128, qoff:qoff + QC], start=True, stop=True)
                        if prev[0] is not None:
                            if kt == 0:
                                drain1(prev[0])
                            elif kt == 1:
                                drain2(prev[0])
                            elif kt == 2:
                                drain3(prev[0])
                                prev[0] = None
                        pt = att.tile([128, 2, QC], BF, tag="pt")
                        e = nc.scalar.activation(pt[:], sc[:], AF.Exp)
                        if sched["bar"] is not None:
                            tile.add_dep_helper(
                                e.ins, sched["bar"].ins,
                                sync=False, reason="act cluster barrier")
                            sched["bar"] = None
                        sched["last"] = e
                        if fi < len(feeder) and feeder[fi][0] <= slot:
                            feeder[fi][1]()
                            fi += 1
                        pend.append((g, pt))
                        pv_flush(False)
                        slot += 1
                    pv_flush(True)
                    prev[0] = [p0, p1, qc]
                # flush remaining feeder + final drain
                while fi < len(feeder):
                    feeder[fi][1]()
                    fi += 1
                drain1(prev[0])
                drain2(prev[0])
                drain3(prev[0])
                return shard

            def reshard(b, shard):
                nc.gpsimd.collective_compute(
                    "AllToAll",
                    mybir.AluOpType.bypass,
                    replica_groups=[list(range(NCORE))],
                    ins=[a2a_in[b][:]],
                    outs=[a2a_out[b][:]],
                )

            def proj_items(b):
                st = {}

                def gather():
                    gat = gatp.tile([128, 8, ST], BF, tag="gat")
                    nc.gpsimd.dma_start(
                        gat[:], a2a_out[b][:].rearrange("c p t -> p c t"))
                    st["g"] = gat

                def mmgroup(tg, d5):
                    gat = st["g"]
                    ps = psml.tile([128, 512], F32, tag="psml")
                    for c in range(8):
                        nc.tensor.matmul(
                            ps[:], gat[:, c, tg * 128:(tg + 1) * 128],
                            wp_sb[:, c, d5 * 512:(d5 + 1) * 512],
                            start=(c == 0), stop=False)
                    nc.tensor.matmul(
                        ps[:], ones1_128[:], bp_sb[:, d5 * 512:(d5 + 1) * 512],
                        start=False, stop=True)
                    ysb = gatp.tile([128, 512], F32, tag="ysb")
                    nc.vector.tensor_copy(ysb[:], ps[:])
                    nc.gpsimd.dma_start(
                        out_d[b, tg * 128:(tg + 1) * 128, d5 * 512:(d5 + 1) * 512],
                        ysb[:])

                return [gather] + [
                    (lambda tg=tg, d5=d5: mmgroup(tg, d5))
                    for tg in range(2) for d5 in range(2)]

            # ---------- main pipeline ----------
            for _s, it in sorted(qkv_norm_items(0), key=lambda x: x[0]):
                it()
            for b in range(B):
                feeder = []
                if b + 1 < B:
                    feeder += qkv_norm_items(b + 1)
                if b >= 1:
                    feeder += [(41 + 2 * i, f) for i, f in enumerate(proj_items(b - 1))]
                feeder.sort(key=lambda x: x[0])
                shard = attention(b, feeder)
                reshard(b, shard)
            for f in proj_items(B - 1):
                f()

    nc.compile()
    _BUILD_CACHE["nc"] = nc
    return nc


def _host_prep(x, rope_cos, rope_sin, w_qkv, w_proj, b_proj, q_norm_w, k_norm_w):
    x = np.asarray(x, np.float32)
    xT = np.ascontiguousarray(
        x.reshape(T, C).T.reshape(8, 128, T).transpose(1, 0, 2)).astype(BF16)
    cosT = np.asarray(rope_cos, np.float32)[0, 0].T          # [hd, N]
    sinT = np.asarray(rope_sin, np.float32)[0, 0].T

    def fold(w, s):
        w = np.asarray(w, np.float32)
        cw = cosT * w[:, None] * s
        sw = np.empty_like(sinT)
        sw[:h2] = -sinT[:h2] * w[h2:HD, None] * s
        sw[h2:] = sinT[h2:] * w[0:h2, None] * s
        dup = lambda a: np.ascontiguousarray(np.concatenate([a, a], 0)).astype(BF16)
        return dup(cw), dup(sw)

    cosq, sinq = fold(q_norm_w, 1.0)     # 8 (rms) * 0.125 (softmax scale) = 1
    cosk, sink = fold(k_norm_w, 8.0)
    w_proj = np.asarray(w_proj, np.float32)
    wp = np.ascontiguousarray(
        w_proj.T.reshape(8, 128, C).transpose(1, 0, 2)).astype(BF16)
    bp = np.asarray(b_proj, np.float32).reshape(1, C).astype(BF16)
    w_qkv = np.asarray(w_qkv, np.float32)

    in_maps = []
    for r in range(NCORE):
        wq = w_qkv[QKCH * r:QKCH * (r + 1), :].T
        wk = w_qkv[C + QKCH * r:C + QKCH * (r + 1), :].T
        wv = w_qkv[2 * C + QKCH * r:2 * C + QKCH * (r + 1), :].T
        wqkvT = np.concatenate([wq, wk, wv], axis=1)         # [C, 384]
        wqkv = np.ascontiguousarray(
            wqkvT.reshape(8, 128, 3 * QKCH).transpose(1, 0, 2)).astype(BF16)
        in_maps.append({
            "xT": xT, "wqkv": wqkv, "wp": wp, "bp": bp,
            "cosq": cosq, "sinq": sinq, "cosk": cosk, "sink": sink,
            "eye": np.eye(128, dtype=BF16),
        })
    return in_maps


def _run(in_maps, trace=False, **kwargs):
    nc = _build()
    return run_bass_kernel_spmd(
        nc, in_maps, core_ids=list(range(NCORE)), trace=trace, **kwargs)


def _unshard(res):
    outs = np.stack(
        [np.asarray(res.results[r]["out"], np.float32) for r in range(NCORE)])
    # outs: [core j, b, 256, C] -> y[b, j*256:(j+1)*256, :]
    return np.ascontiguousarray(outs.transpose(1, 0, 2, 3).reshape(B, N, C))


def kernel(**inputs):
    in_maps = _host_prep(**inputs)
    res = _run(in_maps)
    return _unshard(res)


# revision 50
# speedup vs baseline: 1.4629x; 1.0622x over previous
"""Trainium2 Bass kernel for nn_Attention_22539988369511 (v2).

Dense transformer attention block (B=4, N=2048, C=1024, H=16, hd=64),
sharded over 8 NeuronCores with tensor parallelism over heads (2 heads
per core), per-batch AllToAll with token striping for the output
projection.

Key design points vs v1:
 - Scores for the two heads run CONCURRENTLY in disjoint PE row-groups
   (K=64 each: rows 0-63 and 64-127), writing adjacent PSUM banks, so
   one exp activation covers both heads ([128, 1024] per k-tile).
 - All scale factors fold away: q tables carry w_q (8*0.125=1), k
   tables carry 8*w_k, and the per-token rsqrt factors are multiplied
   into qstore/kstore via K=1 broadcast matmuls, so exp has no scale AP.
 - RMS rsqrt = exp(-0.5*ln(sumsq+eps)) on ACT: ln+exp live in ONE
   activation table set -> zero table switches in the whole kernel.
   Sumsq lands at partitions {0,32,64,96} via mask matmuls so the K=1
   broadcast matmuls stay legal.
 - qkv/norm work for batch b+1 and projection for b-1 are interleaved
   into attention(b)'s PE-queue gaps (attention is ACT-bound).
 - Per-batch AllToAll (token stripes of 256) overlaps collectives with
   compute; each core owns stripe j of every batch for the projection.
"""
import os
import sys

import numpy as np
import ml_dtypes

for _p in ("/opt/trn_rl_repo", "/root/.axon_site/_ro/trn_rl_repo"):
    if os.path.isdir(_p) and _p not in sys.path:
        sys.path.append(_p)

import concourse.bass as bass
import concourse.mybir as mybir
from concourse import bacc, tile
from concourse.bass_utils import run_bass_kernel_spmd

BF16 = ml_dtypes.bfloat16
F32 = mybir.dt.float32
BF = mybir.dt.bfloat16
AF = mybir.ActivationFunctionType

NCORE = 8
B, N, C, H, HD = 4, 2048, 1024, 16, 64
HL = H // NCORE           # 2 heads per core
QKCH = HL * HD            # 128 q (or k) channels per core
T = B * N                 # 8192 tokens
ST = N // NCORE           # 256-token output stripe per core per batch
TOKC = 1024               # qkv/norm token chunk
QC = 512                  # attention q chunk
KT = N // 128             # k tiles per batch (16)
h2 = HD // 2
EPS_SUM = 64.0 * 1e-6     # eps on the 64-element sumsq

_BUILD_CACHE = {}


def _build():
    if "nc" in _BUILD_CACHE:
        return _BUILD_CACHE["nc"]
    nc = bacc.Bacc(None, target_bir_lowering=False, debug=True)

    xT_d = nc.declare_dram_parameter("xT", [128, 8, T], BF, isOutput=False)
    wqkv_d = nc.declare_dram_parameter("wqkv", [128, 8, 3 * QKCH], BF, isOutput=False)
    wp_d = nc.declare_dram_parameter("wp", [128, 8, C], BF, isOutput=False)
    bp_d = nc.declare_dram_parameter("bp", [1, C], BF, isOutput=False)
    cosq_d = nc.declare_dram_parameter("cosq", [128, N], BF, isOutput=False)
    sinq_d = nc.declare_dram_parameter("sinq", [128, N], BF, isOutput=False)
    cosk_d = nc.declare_dram_parameter("cosk", [128, N], BF, isOutput=False)
    sink_d = nc.declare_dram_parameter("sink", [128, N], BF, isOutput=False)
    eye_d = nc.declare_dram_parameter("eye", [128, 128], BF, isOutput=False)
    out_d = nc.declare_dram_parameter("out", [B, ST, C], F32, isOutput=True)

    a2a_in = [nc.dram_tensor(f"a2a_in{b}", [NCORE, QKCH, ST], BF) for b in range(B)]
    a2a_out = [nc.dram_tensor(f"a2a_out{b}", [NCORE, QKCH, ST], BF) for b in range(B)]

    with tile.TileContext(nc) as tc:
        with (
            tc.tile_pool(name="persist", bufs=1) as pp,
            tc.tile_pool(name="xt", bufs=4) as xtp,
            tc.tile_pool(name="nrm", bufs=2) as nrm,
            tc.tile_pool(name="rfp", bufs=4) as rfp,
            tc.tile_pool(name="att", bufs=3) as att,
            tc.tile_pool(name="shp", bufs=2) as shp,
            tc.tile_pool(name="gatp", bufs=2) as gatp,
            tc.tile_pool(name="drp", bufs=2) as drp,
            tc.tile_pool(name="scp", bufs=2, space="PSUM") as scp,
            tc.tile_pool(name="paccp", bufs=2, space="PSUM") as paccp,
            tc.tile_pool(name="psml", bufs=2, space="PSUM") as psml,
        ):
            # ---- resident tiles ----
            w_sb = pp.tile([128, 8, 3 * QKCH], BF)
            wp_sb = pp.tile([128, 8, C], BF)
            bp_sb = pp.tile([1, C], BF)
            rope_sb = pp.tile([128, 4, N], BF)          # cosq|sinq|cosk|sink
            qstore = pp.tile([QKCH, T], BF)
            kstore = pp.tile([QKCH, T], BF)
            # per head: [V (64) | ones (32)] so P@V emits the softmax
            # denominator pre-broadcast across 32 partitions
            VW = HD + 32
            vstore = pp.tile([128, T // 128, 2 * VW], BF)
            eye_sb = pp.tile([128, 128], BF)
            # block-diagonal ones: sumsq matvec output lands pre-broadcast
            # on all 128 partitions (rows 0-63 head0, 64-127 head1)
            maskD = pp.tile([128, 128], BF)
            ones_sb = pp.tile([128, HD], BF)            # K=1 lhsT rows at any partition
            ones1_128 = pp.tile([1, 128], BF)
            eps_col = pp.tile([128, 1], F32)            # rms eps as activation bias

            nc.sync.dma_start(w_sb[:], wqkv_d[:])
            nc.sync.dma_start(wp_sb[:], wp_d[:])
            nc.sync.dma_start(bp_sb[:], bp_d[:])
            nc.sync.dma_start(eye_sb[:], eye_d[:])
            for i, td in enumerate((cosq_d, sinq_d, cosk_d, sink_d)):
                nc.sync.dma_start(rope_sb[:, i, :], td[:])
            nc.vector.memset(maskD[:], 0.0)
            nc.vector.memset(maskD[0:64, 0:64], 1.0)
            nc.vector.memset(maskD[64:128, 64:128], 1.0)
            nc.vector.memset(ones_sb[:], 1.0)
            nc.vector.memset(ones1_128[:], 1.0)
            nc.vector.memset(eps_col[:], EPS_SUM)
            nc.vector.memset(vstore[:, :, HD:VW], 1.0)
            nc.vector.memset(vstore[:, :, VW + HD:2 * VW], 1.0)

            # ---------- qkv + norm for one batch (two 1024-token chunks) ----
            # ACT-table discipline: all 4 Ln calls for the batch execute as
            # one consecutive cluster (one natural_log load), then the
            # exp(-0.5) calls rejoin the attention Exp stream (one exp load).
            # scheduler-order glue so Ln bursts stay contiguous between
            # attention Exps (minimizes ACT table-set reloads)
            sched = {"last": None, "bar": None}

            def _chain(instrs):
                for a, b2 in zip(instrs, instrs[1:]):
                    tile.add_dep_helper(
                        b2.ins, a.ins, sync=False, reason="act table cluster")

            def qkv_norm_items(b):
                items = []
                lns = []        # deferred matvec+ln closures, run as one item
                exps = []       # deferred exp(-0.5) closures
                tail = []       # rope + r-multiply items
                actins = []     # ACT instructions to keep contiguous
                for ti in (2 * b, 2 * b + 1):
                    tok0 = ti * TOKC
                    n0 = tok0 % N
                    st = {}

                    def xload(tok0=tok0, st=st):
                        for t5 in range(2):
                            tk0 = tok0 + t5 * 512
                            xt = xtp.tile([128, 8, 512], BF, tag="xt")
                            nc.sync.dma_start(xt[:], xT_d[:, :, tk0:tk0 + 512])
                            st[t5] = xt

                    def qk_group(t5, m, ti=ti, tok0=tok0, st=st):
                        tk0 = tok0 + t5 * 512
                        xt = st[t5]
                        store = qstore if m == 0 else kstore
                        ps = psml.tile([128, 512], F32, tag="psml")
                        for c in range(8):
                            nc.tensor.matmul(
                                ps[:], w_sb[:, c, m * QKCH:(m + 1) * QKCH],
                                xt[:, c, :], start=(c == 0), stop=(c == 7))
                        nc.vector.tensor_copy(store[:, tk0:tk0 + 512], ps[:])

                    def v_mm(t5, tok0=tok0, st=st):
                        # v as [vch, tok] (weights stationary), then cast to
                        # SBUF for the PE transposes
                        xt = st[t5]
                        ps = psml.tile([128, 512], F32, tag="psml")
                        for c in range(8):
                            nc.tensor.matmul(
                                ps[:], w_sb[:, c, 2 * QKCH:3 * QKCH],
                                xt[:, c, :], start=(c == 0), stop=(c == 7))
                        vtmp = nrm.tile([128, 512], BF, tag="vtmp")
                        nc.vector.tensor_copy(vtmp[:], ps[:])
                        st["v%d" % t5] = vtmp

                    def v_tr(t5, tok0=tok0, st=st):
                        tk0 = tok0 + t5 * 512
                        vtmp = st["v%d" % t5]
                        ps = psml.tile([128, 512], F32, tag="psml")
                        pt4 = ps[:].bitcast(BF)
                        for t1 in range(4):
                            nc.tensor.transpose(
                                pt4[:, t1 * 128:(t1 + 1) * 128],
                                vtmp[:, t1 * 128:(t1 + 1) * 128], eye_sb[:])
                        for t1 in range(4):
                            g = (tk0 // 128) + t1
                            nc.vector.tensor_copy(
                                vstore[:, g, :].rearrange(
                                    "p (a b) -> p a b", b=VW)[:, :, 0:HD],
                                pt4[:, t1 * 128:(t1 + 1) * 128].rearrange(
                                    "p (a b) -> p a b", b=HD))

                    def squares(tok0=tok0, st=st):
                        sqq = nrm.tile([128, TOKC], BF, tag="sqq")
                        sqk = nrm.tile([128, TOKC], BF, tag="sqk")
                        nc.vector.tensor_mul(
                            sqq[:], qstore[:, tok0:tok0 + TOKC], qstore[:, tok0:tok0 + TOKC])
                        nc.vector.tensor_mul(
                            sqk[:], kstore[:, tok0:tok0 + TOKC], kstore[:, tok0:tok0 + TOKC])
                        st["sqq"], st["sqk"] = sqq, sqk

                    def rf_ln(m, ch, st=st):
                        sq = st["sqq"] if m == 0 else st["sqk"]
                        ps = psml.tile([128, 512], F32, tag="psml")
                        nc.tensor.matmul(
                            ps[:], maskD[:], sq[:, ch * 512:(ch + 1) * 512],
                            start=True, stop=True)
                        lnscr = rfp.tile([128, 512], F32, tag="lnscr")
                        actins.append(nc.scalar.activation(
                            lnscr[:], ps[:], AF.Ln, bias=eps_col[:]))
                        st["ln%d%d" % (m, ch)] = lnscr

                    def rf_exp(m, ch, st=st):
                        rall = rfp.tile([128, 512], BF, tag="rall")
                        actins.append(nc.scalar.activation(
                            rall[:], st["ln%d%d" % (m, ch)], AF.Exp, scale=-0.5))
                        st["rall%d%d" % (m, ch)] = rall

                    def rope(m, tok0=tok0, n0=n0, st=st):
                        store = qstore if m == 0 else kstore
                        slf = store[:, tok0:tok0 + TOKC]
                        qrot = nrm.tile([128, TOKC], BF, tag="qrot")
                        for r0 in (0, HD):
                            nc.vector.tensor_copy(
                                qrot[r0:r0 + h2, :], store[r0 + h2:r0 + HD, tok0:tok0 + TOKC])
                            nc.vector.tensor_copy(
                                qrot[r0 + h2:r0 + HD, :], store[r0:r0 + h2, tok0:tok0 + TOKC])
                        cw = rope_sb[:, 2 * m, n0:n0 + TOKC]
                        sw = rope_sb[:, 2 * m + 1, n0:n0 + TOKC]
                        tms = nrm.tile([128, TOKC], BF, tag="tms")
                        nc.vector.tensor_mul(slf, slf, cw)
                        nc.vector.tensor_mul(tms[:], qrot[:], sw)
                        nc.vector.tensor_add(slf, slf, tms[:])

                    def rmul(m, tok0=tok0, st=st):
                        # multiply the pre-broadcast rsqrt into the store
                        store = qstore if m == 0 else kstore
                        for ch in range(2):
                            sl = store[:, tok0 + ch * 512:tok0 + (ch + 1) * 512]
                            nc.vector.tensor_mul(sl, sl, st["rall%d%d" % (m, ch)][:])

                    items.append((xload, squares, qk_group, v_mm, v_tr))
                    lns.append(lambda f=rf_ln: (f(0, 0), f(0, 1), f(1, 0), f(1, 1)))
                    exps.append(lambda f=rf_exp: (f(0, 0), f(0, 1), f(1, 0), f(1, 1)))
                    tail += [
                        lambda f=rope: f(0),
                        lambda f=rmul: f(0),
                        lambda f=rope: f(1),
                        lambda f=rmul: f(1),
                    ]
                def mk_cluster(run):
                    def item():
                        start = len(actins)
                        run()
                        seq = actins[start:]
                        head = sched["last"] or sched["bar"]
                        if head is not None:
                            tile.add_dep_helper(
                                seq[0].ins, head.ins,
                                sync=False, reason="act cluster head")
                        _chain(seq)
                        sched["bar"] = seq[-1]
                    return item

                (xl0, sq0, qk0, vm0, vt0), (xl1, sq1, qk1, vm1, vt1) = items
                # (slot, closure): DMA-only prefetch first so MM groups never
                # head-of-line block the PE queue on an HBM load; one short
                # PE burst per slot so the exp stream never starves
                slotted = [
                    (0, xl0), (1, xl1),
                    (2, lambda: qk0(0, 0)), (3, lambda: qk0(0, 1)),
                    (4, lambda: qk0(1, 0)), (5, lambda: qk0(1, 1)),
                    (6, lambda: qk1(0, 0)), (7, lambda: qk1(0, 1)),
                    (8, lambda: qk1(1, 0)), (9, lambda: qk1(1, 1)),
                    (10, sq0),
                    (11, lambda: vm0(0)), (12, lambda: vt0(0)),
                    (13, lambda: vm0(1)), (14, lambda: vt0(1)),
                    (15, sq1),
                    (16, lambda: vm1(0)), (17, lambda: vt1(0)),
                    (18, lambda: vm1(1)), (19, lambda: vt1(1)),
                    (20, mk_cluster(lns[0])),
                    (22, mk_cluster(exps[0])),
                    (24, mk_cluster(lns[1])),
                    (26, mk_cluster(exps[1])),
                ]
                slotted += [(28 + 2 * i, f) for i, f in enumerate(tail)]
                return slotted

            # ---------- attention for one batch ----------
            def attention(b, feeder):
                boff = b * N
                shard = shp.tile([QKCH, N], BF, tag="shard")
                fi = 0
                slot = 0
                prev = [None]

                def drain1(pr):
                    p0, p1, qc = pr
                    den0 = drp.tile([32, QC], F32, tag="den0")
                    den1 = drp.tile([32, QC], F32, tag="den1")
                    nc.vector.tensor_copy(den0[:], p0[64:96, :])
                    nc.vector.tensor_copy(den1[:], p1[64:96, :])
                    pvs = drp.tile([128, QC], BF, tag="pvs")
                    nc.vector.tensor_copy(pvs[0:64, :], p0[0:64, :])
                    nc.vector.tensor_copy(pvs[64:128, :], p1[0:64, :])
                    pr += [den0, den1, pvs]

                def drain2(pr):
                    den0, den1 = pr[3], pr[4]
                    drec0 = drp.tile([32, QC], F32, tag="drec0")
                    drec1 = drp.tile([32, QC], F32, tag="drec1")
                    with nc.allow_low_precision(reason="softmax denom"):
                        nc.vector.reciprocal_approx_fast(drec0[:], den0[:])
                        nc.vector.reciprocal_approx_fast(drec1[:], den1[:])
                    d01 = drp.tile([128, QC], BF, tag="d01")
                    nc.vector.tensor_copy(d01[0:32, :], drec0[:])
                    nc.vector.tensor_copy(d01[32:64, :], drec0[:])
                    nc.vector.tensor_copy(d01[64:96, :], drec1[:])
                    nc.vector.tensor_copy(d01[96:128, :], drec1[:])
                    pr.append(d01)

                def drain3(pr):
                    _p0, _p1, qc, _d0f, _d1f, pvs, d01 = pr
                    nc.vector.tensor_mul(
                        shard[:, qc * QC:(qc + 1) * QC], pvs[:], d01[:])
                    # stage this qc's two 256-token stripes for the AllToAll;
                    # keep it off the Sync queue so it never queues behind
                    # x-loads
                    nc.gpsimd.dma_start(
                        a2a_in[b][2 * qc:2 * qc + 2].rearrange("j p t -> p j t"),
                        shard[:, qc * QC:(qc + 1) * QC].rearrange(
                            "p (j t) -> p j t", j=2))

                for qc in range(N // QC):
                    qoff = boff + qc * QC
                    p0 = paccp.tile([96, QC], F32, tag="pacc")
                    p1 = paccp.tile([96, QC], F32, tag="pacc")
                    pend = []      # pts awaiting PV, two k-tiles behind exp

                    def pv_flush(last):
                        while pend and (len(pend) > 2 or last):
                            gp, ptp = pend.pop(0)
                            nc.tensor.matmul(
                                p0[:], vstore[:, gp, 0:VW], ptp[:, 0, :],
                                start=(gp % KT == 0), stop=(last and not pend))
                            nc.tensor.matmul(
                                p1[:], vstore[:, gp, VW:2 * VW], ptp[:, 1, :],
                                start=(gp % KT == 0), stop=(last and not pend))

                    for kt in range(KT):
                        koff = boff + kt * 128
                        g = koff // 128
                        sc = scp.tile([128, 2, QC], F32, tag="sc")
                        nc.tensor.matmul(
                            sc[:, 0, :], kstore[0:HD, koff:koff + 128],
                            qstore[0:HD, qoff:qoff + QC], start=True, stop=True)
                        nc.tensor.matmul(
                            sc[:, 1, :], kstore[HD:128, koff:koff + 128],
                            qstore[HD:128, qoff:qoff + QC], start=True, stop=True)
                        if prev[0] is not None:
                            if kt == 0:
                                drain1(prev[0])
                            elif kt == 1:
                                drain2(prev[0])
                            elif kt == 2:
                                drain3(prev[0])
                                prev[0] = None
                        pt = att.tile([128, 2, QC], BF, tag="pt")
                        e = nc.scalar.activation(pt[:], sc[:], AF.Exp)
                        if sched["bar"] is not None:
                            tile.add_dep_helper(
                                e.ins, sched["bar"].ins,
                                sync=False, reason="act cluster barrier")
                            sched["bar"] = None
                        sched["last"] = e
                        if fi < len(feeder) and feeder[fi][0] <= slot:
                            feeder[fi][1]()
                            fi += 1
                        pend.append((g, pt))
                        pv_flush(False)
                        slot += 1
                    pv_flush(True)
                    prev[0] = [p0, p1, qc]
                # flush remaining feeder + final drain
                while fi < len(feeder):
                    feeder[fi][1]()
                    fi += 1
                drain1(prev[0])
                drain2(prev[0])
                drain3(prev[0])
                return shard

            def reshard(b, shard):
                nc.gpsimd.collective_compute(
                    "AllToAll",
                    mybir.AluOpType.bypass,
                    replica_groups=[list(range(NCORE))],
                    ins=[a2a_in[b][:]],
                    outs=[a2a_out[b][:]],
                )

            def proj_items(b):
                st = {}

                def gather():
                    gat = gatp.tile([128, 8, ST], BF, tag="gat")
                    nc.gpsimd.dma_start(
                        gat[:], a2a_out[b][:].rearrange("c p t -> p c t"))
                    st["g"] = gat

                def mmgroup(tg, d5):
                    gat = st["g"]
                    ps = psml.tile([128, 512], F32, tag="psml")
                    for c in range(8):
                        nc.tensor.matmul(
                            ps[:], gat[:, c, tg * 128:(tg + 1) * 128],
                            wp_sb[:, c, d5 * 512:(d5 + 1) * 512],
                            start=(c == 0), stop=False)
                    nc.tensor.matmul(
                        ps[:], ones1_128[:], bp_sb[:, d5 * 512:(d5 + 1) * 512],
                        start=False, stop=True)
                    ysb = gatp.tile([128, 512], F32, tag="ysb")
                    nc.vector.tensor_copy(ysb[:], ps[:])
                    nc.gpsimd.dma_start(
                        out_d[b, tg * 128:(tg + 1) * 128, d5 * 512:(d5 + 1) * 512],
                        ysb[:])

                return [gather] + [
                    (lambda tg=tg, d5=d5: mmgroup(tg, d5))
                    for tg in range(2) for d5 in range(2)]

            # ---------- main pipeline ----------
            for _s, it in sorted(qkv_norm_items(0), key=lambda x: x[0]):
                it()
            for b in range(B):
                feeder = []
                if b + 1 < B:
                    feeder += qkv_norm_items(b + 1)
                if b >= 1:
                    pj = proj_items(b - 1)
                    feeder += [(20, pj[0])] + [
                        (29 + 4 * i, f) for i, f in enumerate(pj[1:])]
                feeder.sort(key=lambda x: x[0])
                shard = attention(b, feeder)
                reshard(b, shard)
            for f in proj_items(B - 1):
                f()

    nc.compile()
    _BUILD_CACHE["nc"] = nc
    return nc


def _host_prep(x, rope_cos, rope_sin, w_qkv, w_proj, b_proj, q_norm_w, k_norm_w):
    x = np.asarray(x, np.float32)
    xT = np.ascontiguousarray(
        x.reshape(T, C).T.reshape(8, 128, T).transpose(1, 0, 2)).astype(BF16)
    cosT = np.asarray(rope_cos, np.float32)[0, 0].T          # [hd, N]
    sinT = np.asarray(rope_sin, np.float32)[0, 0].T

    def fold(w, s):
        w = np.asarray(w, np.float32)
        cw = cosT * w[:, None] * s
        sw = np.empty_like(sinT)
        sw[:h2] = -sinT[:h2] * w[h2:HD, None] * s
        sw[h2:] = sinT[h2:] * w[0:h2, None] * s
        dup = lambda a: np.ascontiguousarray(np.concatenate([a, a], 0)).astype(BF16)
        return dup(cw), dup(sw)

    cosq, sinq = fold(q_norm_w, 1.0)     # 8 (rms) * 0.125 (softmax scale) = 1
    cosk, sink = fold(k_norm_w, 8.0)
    w_proj = np.asarray(w_proj, np.float32)
    wp = np.ascontiguousarray(
        w_proj.T.reshape(8, 128, C).transpose(1, 0, 2)).astype(BF16)
    bp = np.asarray(b_proj, np.float32).reshape(1, C).astype(BF16)
    w_qkv = np.asarray(w_qkv, np.float32)

    in_maps = []
    for r in range(NCORE):
        wq = w_qkv[QKCH * r:QKCH * (r + 1), :].T
        wk = w_qkv[C + QKCH * r:C + QKCH * (r + 1), :].T
        wv = w_qkv[2 * C + QKCH * r:2 * C + QKCH * (r + 1), :].T
        wqkvT = np.concatenate([wq, wk, wv], axis=1)         # [C, 384]
        wqkv = np.ascontiguousarray(
            wqkvT.reshape(8, 128, 3 * QKCH).transpose(1, 0, 2)).astype(BF16)
        in_maps.append({
            "xT": xT, "wqkv": wqkv, "wp": wp, "bp": bp,
            "cosq": cosq, "sinq": sinq, "cosk": cosk, "sink": sink,
            "eye": np.eye(128, dtype=BF16),
        })
    return in_maps


def _run(in_maps, trace=False, **kwargs):
    nc = _build()
    return run_bass_kernel_spmd(
        nc, in_maps, core_ids=list(range(NCORE)), trace=trace, **kwargs)


def _unshard(res):
    outs = np.stack(
        [np.asarray(res.results[r]["out"], np.float32) for r in range(NCORE)])
    # outs: [core j, b, 256, C] -> y[b, j*256:(j+1)*256, :]
    return np.ascontiguousarray(outs.transpose(1, 0, 2, 3).reshape(B, N, C))


def kernel(**inputs):
    in_maps = _host_prep(**inputs)
    res = _run(in_maps)
    return _unshard(res)


# revision 55
# speedup vs baseline: 1.4647x; 1.0012x over previous
"""Trainium2 Bass kernel for nn_Attention_22539988369511 (v2).

Dense transformer attention block (B=4, N=2048, C=1024, H=16, hd=64),
sharded over 8 NeuronCores with tensor parallelism over heads (2 heads
per core), per-batch AllToAll with token striping for the output
projection.

Key design points vs v1:
 - Scores for the two heads run CONCURRENTLY in disjoint PE row-groups
   (K=64 each: rows 0-63 and 64-127), writing adjacent PSUM banks, so
   one exp activation covers both heads ([128, 1024] per k-tile).
 - All scale factors fold away: q tables carry w_q (8*0.125=1), k
   tables carry 8*w_k, and the per-token rsqrt factors are multiplied
   into qstore/kstore via K=1 broadcast matmuls, so exp has no scale AP.
 - RMS rsqrt = exp(-0.5*ln(sumsq+eps)) on ACT: ln+exp live in ONE
   activation table set -> zero table switches in the whole kernel.
   Sumsq lands at partitions {0,32,64,96} via mask matmuls so the K=1
   broadcast matmuls stay legal.
 - qkv/norm work for batch b+1 and projection for b-1 are interleaved
   into attention(b)'s PE-queue gaps (attention is ACT-bound).
 - Per-batch AllToAll (token stripes of 256) overlaps collectives with
   compute; each core owns stripe j of every batch for the projection.
"""
import os
import sys

import numpy as np
import ml_dtypes

for _p in ("/opt/trn_rl_repo", "/root/.axon_site/_ro/trn_rl_repo"):
    if os.path.isdir(_p) and _p not in sys.path:
        sys.path.append(_p)

import concourse.bass as bass
import concourse.mybir as mybir
from concourse import bacc, tile
from concourse.bass_utils import run_bass_kernel_spmd

BF16 = ml_dtypes.bfloat16
F32 = mybir.dt.float32
BF = mybir.dt.bfloat16
AF = mybir.ActivationFunctionType

NCORE = 8
B, N, C, H, HD = 4, 2048, 1024, 16, 64
HL = H // NCORE           # 2 heads per core
QKCH = HL * HD            # 128 q (or k) channels per core
T = B * N                 # 8192 tokens
ST = N // NCORE           # 256-token output stripe per core per batch
TOKC = 1024               # qkv/norm token chunk
QC = 512                  # attention q chunk
KT = N // 128             # k tiles per batch (16)
h2 = HD // 2
EPS_SUM = 64.0 * 1e-6     # eps on the 64-element sumsq

_BUILD_CACHE = {}


def _build():
    if "nc" in _BUILD_CACHE:
        return _BUILD_CACHE["nc"]
    nc = bacc.Bacc(None, target_bir_lowering=False, debug=True)

    xT_d = nc.declare_dram_parameter("xT", [128, 8, T], BF, isOutput=False)
    wqkv_d = nc.declare_dram_parameter("wqkv", [128, 8, 3 * QKCH], BF, isOutput=False)
    wp_d = nc.declare_dram_parameter("wp", [128, 8, C], BF, isOutput=False)
    bp_d = nc.declare_dram_parameter("bp", [1, C], BF, isOutput=False)
    cosq_d = nc.declare_dram_parameter("cosq", [128, N], BF, isOutput=False)
    sinq_d = nc.declare_dram_parameter("sinq", [128, N], BF, isOutput=False)
    cosk_d = nc.declare_dram_parameter("cosk", [128, N], BF, isOutput=False)
    sink_d = nc.declare_dram_parameter("sink", [128, N], BF, isOutput=False)
    eye_d = nc.declare_dram_parameter("eye", [128, 128], BF, isOutput=False)
    out_d = nc.declare_dram_parameter("out", [B, ST, C], F32, isOutput=True)

    a2a_in = [nc.dram_tensor(f"a2a_in{b}", [NCORE, QKCH, ST], BF) for b in range(B)]
    a2a_out = [nc.dram_tensor(f"a2a_out{b}", [NCORE, QKCH, ST], BF) for b in range(B)]

    with tile.TileContext(nc) as tc:
        with (
            tc.tile_pool(name="persist", bufs=1) as pp,
            tc.tile_pool(name="xt", bufs=4) as xtp,
            tc.tile_pool(name="nrm", bufs=2) as nrm,
            tc.tile_pool(name="rfp", bufs=4) as rfp,
            tc.tile_pool(name="att", bufs=3) as att,
            tc.tile_pool(name="shp", bufs=2) as shp,
            tc.tile_pool(name="gatp", bufs=2) as gatp,
            tc.tile_pool(name="drp", bufs=2) as drp,
            tc.tile_pool(name="scp", bufs=2, space="PSUM") as scp,
            tc.tile_pool(name="paccp", bufs=2, space="PSUM") as paccp,
            tc.tile_pool(name="psml", bufs=2, space="PSUM") as psml,
        ):
            # ---- resident tiles ----
            w_sb = pp.tile([128, 8, 3 * QKCH], BF)
            wp_sb = pp.tile([128, 8, C], BF)
            bp_sb = pp.tile([1, C], BF)
            rope_sb = pp.tile([128, 4, N], BF)          # cosq|sinq|cosk|sink
            qstore = pp.tile([QKCH, T], BF)
            kstore = pp.tile([QKCH, T], BF)
            # per head: [V (64) | ones (32)] so P@V emits the softmax
            # denominator pre-broadcast across 32 partitions
            VW = HD + 32
            vstore = pp.tile([128, T // 128, 2 * VW], BF)
            eye_sb = pp.tile([128, 128], BF)
            # block-diagonal ones: sumsq matvec output lands pre-broadcast
            # on all 128 partitions (rows 0-63 head0, 64-127 head1)
            maskD = pp.tile([128, 128], BF)
            ones_sb = pp.tile([128, HD], BF)            # K=1 lhsT rows at any partition
            ones1_128 = pp.tile([1, 128], BF)
            eps_col = pp.tile([128, 1], F32)            # rms eps as activation bias

            nc.sync.dma_start(w_sb[:], wqkv_d[:])

            def load_late_weights():
                # w_proj is first needed by proj(0) during attention(1);
                # keep its 2MB off the startup critical path
                nc.sync.dma_start(wp_sb[:], wp_d[:])
                nc.sync.dma_start(bp_sb[:], bp_d[:])

            def load_tables():
                nc.sync.dma_start(eye_sb[:], eye_d[:])
                for i, td in enumerate((cosq_d, sinq_d, cosk_d, sink_d)):
                    nc.sync.dma_start(rope_sb[:, i, :], td[:])
            nc.vector.memset(maskD[:], 0.0)
            nc.vector.memset(maskD[0:64, 0:64], 1.0)
            nc.vector.memset(maskD[64:128, 64:128], 1.0)
            nc.vector.memset(ones_sb[:], 1.0)
            nc.vector.memset(ones1_128[:], 1.0)
            nc.vector.memset(eps_col[:], EPS_SUM)
            nc.vector.memset(vstore[:, :, HD:VW], 1.0)
            nc.vector.memset(vstore[:, :, VW + HD:2 * VW], 1.0)

            # ---------- qkv + norm for one batch (two 1024-token chunks) ----
            # ACT-table discipline: all 4 Ln calls for the batch execute as
            # one consecutive cluster (one natural_log load), then the
            # exp(-0.5) calls rejoin the attention Exp stream (one exp load).
            # scheduler-order glue so Ln bursts stay contiguous between
            # attention Exps (minimizes ACT table-set reloads)
            sched = {"last": None, "bar": None}

            def _chain(instrs):
                for a, b2 in zip(instrs, instrs[1:]):
                    tile.add_dep_helper(
                        b2.ins, a.ins, sync=False, reason="act table cluster")

            def qkv_norm_items(b):
                items = []
                lns = []        # deferred matvec+ln closures, run as one item
                exps = []       # deferred exp(-0.5) closures
                tail = []       # rope + r-multiply items
                actins = []     # ACT instructions to keep contiguous
                for ti in (2 * b, 2 * b + 1):
                    tok0 = ti * TOKC
                    n0 = tok0 % N
                    st = {}

                    def xload(tok0=tok0, st=st):
                        for t5 in range(2):
                            tk0 = tok0 + t5 * 512
                            xt = xtp.tile([128, 8, 512], BF, tag="xt")
                            nc.sync.dma_start(xt[:], xT_d[:, :, tk0:tk0 + 512])
                            st[t5] = xt

                    def qk_group(t5, m, ti=ti, tok0=tok0, st=st):
                        tk0 = tok0 + t5 * 512
                        xt = st[t5]
                        store = qstore if m == 0 else kstore
                        ps = psml.tile([128, 512], F32, tag="psml")
                        for c in range(8):
                            nc.tensor.matmul(
                                ps[:], w_sb[:, c, m * QKCH:(m + 1) * QKCH],
                                xt[:, c, :], start=(c == 0), stop=(c == 7))
                        nc.vector.tensor_copy(store[:, tk0:tk0 + 512], ps[:])

                    def v_mm(t5, tok0=tok0, st=st):
                        # v as [vch, tok] (weights stationary), then cast to
                        # SBUF for the PE transposes
                        xt = st[t5]
                        ps = psml.tile([128, 512], F32, tag="psml")
                        for c in range(8):
                            nc.tensor.matmul(
                                ps[:], w_sb[:, c, 2 * QKCH:3 * QKCH],
                                xt[:, c, :], start=(c == 0), stop=(c == 7))
                        vtmp = nrm.tile([128, 512], BF, tag="vtmp")
                        nc.vector.tensor_copy(vtmp[:], ps[:])
                        st["v%d" % t5] = vtmp

                    def v_tr(t5, tok0=tok0, st=st):
                        tk0 = tok0 + t5 * 512
                        vtmp = st["v%d" % t5]
                        ps = psml.tile([128, 512], F32, tag="psml")
                        pt4 = ps[:].bitcast(BF)
                        for t1 in range(4):
                            nc.tensor.transpose(
                                pt4[:, t1 * 128:(t1 + 1) * 128],
                                vtmp[:, t1 * 128:(t1 + 1) * 128], eye_sb[:])
                        for t1 in range(4):
                            g = (tk0 // 128) + t1
                            nc.vector.tensor_copy(
                                vstore[:, g, :].rearrange(
                                    "p (a b) -> p a b", b=VW)[:, :, 0:HD],
                                pt4[:, t1 * 128:(t1 + 1) * 128].rearrange(
                                    "p (a b) -> p a b", b=HD))

                    def squares(tok0=tok0, st=st):
                        sqq = nrm.tile([128, TOKC], BF, tag="sqq")
                        sqk = nrm.tile([128, TOKC], BF, tag="sqk")
                        nc.vector.tensor_mul(
                            sqq[:], qstore[:, tok0:tok0 + TOKC], qstore[:, tok0:tok0 + TOKC])
                        nc.vector.tensor_mul(
                            sqk[:], kstore[:, tok0:tok0 + TOKC], kstore[:, tok0:tok0 + TOKC])
                        st["sqq"], st["sqk"] = sqq, sqk

                    def rf_ln(m, ch, st=st):
                        sq = st["sqq"] if m == 0 else st["sqk"]
                        ps = psml.tile([128, 512], F32, tag="psml")
                        nc.tensor.matmul(
                            ps[:], maskD[:], sq[:, ch * 512:(ch + 1) * 512],
                            start=True, stop=True)
                        lnscr = rfp.tile([128, 512], F32, tag="lnscr")
                        actins.append(nc.scalar.activation(
                            lnscr[:], ps[:], AF.Ln, bias=eps_col[:]))
                        st["ln%d%d" % (m, ch)] = lnscr

                    def rf_exp(m, ch, st=st):
                        rall = rfp.tile([128, 512], BF, tag="rall")
                        actins.append(nc.scalar.activation(
                            rall[:], st["ln%d%d" % (m, ch)], AF.Exp, scale=-0.5))
                        st["rall%d%d" % (m, ch)] = rall

                    def rope(m, tok0=tok0, n0=n0, st=st):
                        store = qstore if m == 0 else kstore
                        slf = store[:, tok0:tok0 + TOKC]
                        qrot = nrm.tile([128, TOKC], BF, tag="qrot")
                        for r0 in (0, HD):
                            nc.vector.tensor_copy(
                                qrot[r0:r0 + h2, :], store[r0 + h2:r0 + HD, tok0:tok0 + TOKC])
                            nc.vector.tensor_copy(
                                qrot[r0 + h2:r0 + HD, :], store[r0:r0 + h2, tok0:tok0 + TOKC])
                        cw = rope_sb[:, 2 * m, n0:n0 + TOKC]
                        sw = rope_sb[:, 2 * m + 1, n0:n0 + TOKC]
                        tms = nrm.tile([128, TOKC], BF, tag="tms")
                        nc.vector.tensor_mul(slf, slf, cw)
                        nc.vector.tensor_mul(tms[:], qrot[:], sw)
                        nc.vector.tensor_add(slf, slf, tms[:])

                    def rmul(m, tok0=tok0, st=st):
                        # multiply the pre-broadcast rsqrt into the store
                        store = qstore if m == 0 else kstore
                        for ch in range(2):
                            sl = store[:, tok0 + ch * 512:tok0 + (ch + 1) * 512]
                            nc.vector.tensor_mul(sl, sl, st["rall%d%d" % (m, ch)][:])

                    items.append((xload, squares, qk_group, v_mm, v_tr))
                    lns.append(lambda f=rf_ln: (f(0, 0), f(0, 1), f(1, 0), f(1, 1)))
                    exps.append(lambda f=rf_exp: (f(0, 0), f(0, 1), f(1, 0), f(1, 1)))
                    tail += [
                        lambda f=rope: f(0),
                        lambda f=rmul: f(0),
                        lambda f=rope: f(1),
                        lambda f=rmul: f(1),
                    ]
                def mk_cluster(run):
                    def item():
                        start = len(actins)
                        run()
                        seq = actins[start:]
                        head = sched["last"] or sched["bar"]
                        if head is not None:
                            tile.add_dep_helper(
                                seq[0].ins, head.ins,
                                sync=False, reason="act cluster head")
                        _chain(seq)
                        sched["bar"] = seq[-1]
                    return item

                (xl0, sq0, qk0, vm0, vt0), (xl1, sq1, qk1, vm1, vt1) = items
                # (slot, closure): DMA-only prefetch first so MM groups never
                # head-of-line block the PE queue on an HBM load; one short
                # PE burst per slot so the exp stream never starves
                slotted = [
                    (0, xl0), (1, xl1),
                    (2, lambda: qk0(0, 0)), (3, lambda: qk0(0, 1)),
                    (4, lambda: qk0(1, 0)), (5, lambda: qk0(1, 1)),
                    (6, lambda: qk1(0, 0)), (7, lambda: qk1(0, 1)),
                    (8, lambda: qk1(1, 0)), (9, lambda: qk1(1, 1)),
                    (10, sq0),
                    (11, lambda: vm0(0)), (12, lambda: vt0(0)),
                    (13, lambda: vm0(1)), (14, lambda: vt0(1)),
                    (15, sq1),
                    (16, lambda: vm1(0)), (17, lambda: vt1(0)),
                    (18, lambda: vm1(1)), (19, lambda: vt1(1)),
                    (20, mk_cluster(lns[0])),
                    (22, mk_cluster(exps[0])),
                    (24, mk_cluster(lns[1])),
                    (26, mk_cluster(exps[1])),
                ]
                slotted += [(28 + 2 * i, f) for i, f in enumerate(tail)]
                return slotted

            # ---------- attention for one batch ----------
            def attention(b, feeder):
                boff = b * N
                shard = shp.tile([QKCH, N], BF, tag="shard")
                fi = 0
                slot = 0
                prev = [None]

                def drain1(pr):
                    p0, p1, qc = pr
                    den0 = drp.tile([32, QC], F32, tag="den0")
                    den1 = drp.tile([32, QC], F32, tag="den1")
                    nc.vector.tensor_copy(den0[:], p0[64:96, :])
                    nc.vector.tensor_copy(den1[:], p1[64:96, :])
                    pvs = drp.tile([128, QC], BF, tag="pvs")
                    nc.vector.tensor_copy(pvs[0:64, :], p0[0:64, :])
                    nc.vector.tensor_copy(pvs[64:128, :], p1[0:64, :])
                    pr += [den0, den1, pvs]

                def drain2(pr):
                    den0, den1 = pr[3], pr[4]
                    drec0 = drp.tile([32, QC], F32, tag="drec0")
                    drec1 = drp.tile([32, QC], F32, tag="drec1")
                    with nc.allow_low_precision(reason="softmax denom"):
                        nc.vector.reciprocal_approx_fast(drec0[:], den0[:])
                        nc.vector.reciprocal_approx_fast(drec1[:], den1[:])
                    d01 = drp.tile([128, QC], BF, tag="d01")
                    nc.vector.tensor_copy(d01[0:32, :], drec0[:])
                    nc.vector.tensor_copy(d01[32:64, :], drec0[:])
                    nc.vector.tensor_copy(d01[64:96, :], drec1[:])
                    nc.vector.tensor_copy(d01[96:128, :], drec1[:])
                    pr.append(d01)

                def drain3(pr):
                    _p0, _p1, qc, _d0f, _d1f, pvs, d01 = pr
                    nc.vector.tensor_mul(
                        shard[:, qc * QC:(qc + 1) * QC], pvs[:], d01[:])
                    # stage this qc's two 256-token stripes for the AllToAll;
                    # keep it off the Sync queue so it never queues behind
                    # x-loads
                    nc.gpsimd.dma_start(
                        a2a_in[b][2 * qc:2 * qc + 2].rearrange("j p t -> p j t"),
                        shard[:, qc * QC:(qc + 1) * QC].rearrange(
                            "p (j t) -> p j t", j=2))

                for qc in range(N // QC):
                    qoff = boff + qc * QC
                    p0 = paccp.tile([96, QC], F32, tag="pacc")
                    p1 = paccp.tile([96, QC], F32, tag="pacc")
                    pend = []      # pts awaiting PV, two k-tiles behind exp

                    def pv_flush(last):
                        while pend and (len(pend) > 2 or last):
                            gp, ptp = pend.pop(0)
                            nc.tensor.matmul(
                                p0[:], vstore[:, gp, 0:VW], ptp[:, 0, :],
                                start=(gp % KT == 0), stop=(last and not pend))
                            nc.tensor.matmul(
                                p1[:], vstore[:, gp, VW:2 * VW], ptp[:, 1, :],
                                start=(gp % KT == 0), stop=(last and not pend))

                    for kt in range(KT):
                        koff = boff + kt * 128
                        g = koff // 128
                        # feeder work goes ahead of the score MMs: it fills
                        # the PE-queue wait on the score-buffer WAR
                        if fi < len(feeder) and feeder[fi][0] <= slot:
                            feeder[fi][1]()
                            fi += 1
                        sc = scp.tile([128, 2, QC], F32, tag="sc")
                        nc.tensor.matmul(
                            sc[:, 0, :], kstore[0:HD, koff:koff + 128],
                            qstore[0:HD, qoff:qoff + QC], start=True, stop=True)
                        nc.tensor.matmul(
                            sc[:, 1, :], kstore[HD:128, koff:koff + 128],
                            qstore[HD:128, qoff:qoff + QC], start=True, stop=True)
                        if prev[0] is not None:
                            if kt == 0:
                                drain1(prev[0])
                            elif kt == 1:
                                drain2(prev[0])
                            elif kt == 2:
                                drain3(prev[0])
                                prev[0] = None
                        pt = att.tile([128, 2, QC], BF, tag="pt")
                        e = nc.scalar.activation(pt[:], sc[:], AF.Exp)
                        if sched["bar"] is not None:
                            tile.add_dep_helper(
                                e.ins, sched["bar"].ins,
                                sync=False, reason="act cluster barrier")
                            sched["bar"] = None
                        sched["last"] = e
                        pend.append((g, pt))
                        pv_flush(False)
                        slot += 1
                    pv_flush(True)
                    prev[0] = [p0, p1, qc]
                # flush remaining feeder + final drain
                while fi < len(feeder):
                    feeder[fi][1]()
                    fi += 1
                drain1(prev[0])
                drain2(prev[0])
                drain3(prev[0])
                return shard

            def reshard(b, shard):
                nc.gpsimd.collective_compute(
                    "AllToAll",
                    mybir.AluOpType.bypass,
                    replica_groups=[list(range(NCORE))],
                    ins=[a2a_in[b][:]],
                    outs=[a2a_out[b][:]],
                )

            def proj_items(b):
                st = {}

                def gather():
                    gat = gatp.tile([128, 8, ST], BF, tag="gat")
                    nc.gpsimd.dma_start(
                        gat[:], a2a_out[b][:].rearrange("c p t -> p c t"))
                    st["g"] = gat

                def mmgroup(tg, d5):
                    gat = st["g"]
                    ps = psml.tile([128, 512], F32, tag="psml")
                    for c in range(8):
                        nc.tensor.matmul(
                            ps[:], gat[:, c, tg * 128:(tg + 1) * 128],
                            wp_sb[:, c, d5 * 512:(d5 + 1) * 512],
                            start=(c == 0), stop=(c == 7))
                    ysb = gatp.tile([128, 512], F32, tag="ysb")
                    nc.vector.tensor_copy(ysb[:], ps[:])
                    nc.gpsimd.dma_start(
                        out_d[b, tg * 128:(tg + 1) * 128, d5 * 512:(d5 + 1) * 512],
                        ysb[:])

                return [gather] + [
                    (lambda tg=tg, d5=d5: mmgroup(tg, d5))
                    for tg in range(2) for d5 in range(2)]

            # ---------- main pipeline ----------
            items0 = sorted(qkv_norm_items(0), key=lambda x: x[0])
            items0[0][1]()          # x prefetch right behind the qkv weights
            items0[1][1]()
            load_tables()
            for _s, it in items0[2:]:
                it()
            for b in range(B):
                feeder = []
                if b + 1 < B:
                    feeder += qkv_norm_items(b + 1)
                if b == 0:
                    feeder.append((27, load_late_weights))
                if b >= 1:
                    pj = proj_items(b - 1)
                    feeder += [(20, pj[0])] + [
                        (29 + 4 * i, f) for i, f in enumerate(pj[1:])]
                feeder.sort(key=lambda x: x[0])
                shard = attention(b, feeder)
                reshard(b, shard)
            for f in proj_items(B - 1):
                f()

    nc.compile()
    _BUILD_CACHE["nc"] = nc
    return nc


def _host_prep(x, rope_cos, rope_sin, w_qkv, w_proj, b_proj, q_norm_w, k_norm_w):
    x = np.asarray(x, np.float32)
    xT = np.ascontiguousarray(
        x.reshape(T, C).T.reshape(8, 128, T).transpose(1, 0, 2)).astype(BF16)
    cosT = np.asarray(rope_cos, np.float32)[0, 0].T          # [hd, N]
    sinT = np.asarray(rope_sin, np.float32)[0, 0].T

    def fold(w, s):
        w = np.asarray(w, np.float32)
        cw = cosT * w[:, None] * s
        sw = np.empty_like(sinT)
        sw[:h2] = -sinT[:h2] * w[h2:HD, None] * s
        sw[h2:] = sinT[h2:] * w[0:h2, None] * s
        dup = lambda a: np.ascontiguousarray(np.concatenate([a, a], 0)).astype(BF16)
        return dup(cw), dup(sw)

    cosq, sinq = fold(q_norm_w, 1.0)     # 8 (rms) * 0.125 (softmax scale) = 1
    cosk, sink = fold(k_norm_w, 8.0)
    w_proj = np.asarray(w_proj, np.float32)
    wp = np.ascontiguousarray(
        w_proj.T.reshape(8, 128, C).transpose(1, 0, 2)).astype(BF16)
    bp = np.asarray(b_proj, np.float32).reshape(1, C).astype(BF16)
    w_qkv = np.asarray(w_qkv, np.float32)

    in_maps = []
    for r in range(NCORE):
        wq = w_qkv[QKCH * r:QKCH * (r + 1), :].T
        wk = w_qkv[C + QKCH * r:C + QKCH * (r + 1), :].T
        wv = w_qkv[2 * C + QKCH * r:2 * C + QKCH * (r + 1), :].T
        wqkvT = np.concatenate([wq, wk, wv], axis=1)         # [C, 384]
        wqkv = np.ascontiguousarray(
            wqkvT.reshape(8, 128, 3 * QKCH).transpose(1, 0, 2)).astype(BF16)
        in_maps.append({
            "xT": xT, "wqkv": wqkv, "wp": wp, "bp": bp,
            "cosq": cosq, "sinq": sinq, "cosk": cosk, "sink": sink,
            "eye": np.eye(128, dtype=BF16),
        })
    return in_maps


def _run(in_maps, trace=False, **kwargs):
    nc = _build()
    return run_bass_kernel_spmd(
        nc, in_maps, core_ids=list(range(NCORE)), trace=trace, **kwargs)


def _unshard(res):
    outs = np.stack(
        [np.asarray(res.results[r]["out"], np.float32) for r in range(NCORE)])
    # outs: [core j, b, 256, C] -> y[b, j*256:(j+1)*256, :]
    return np.ascontiguousarray(outs.transpose(1, 0, 2, 3).reshape(B, N, C))


def kernel(**inputs):
    in_maps = _host_prep(**inputs)
    res = _run(in_maps)
    y = _unshard(res)
    # projection bias applied on host (zeros by spec, but honor it anyway)
    return y + np.asarray(inputs["b_proj"], np.float32)[None, None, :]


# revision 62
# speedup vs baseline: 1.5246x; 1.0409x over previous
"""Trainium2 Bass kernel for nn_Attention_22539988369511 (v2).

Dense transformer attention block (B=4, N=2048, C=1024, H=16, hd=64),
sharded over 8 NeuronCores with tensor parallelism over heads (2 heads
per core), per-batch AllToAll with token striping for the output
projection.

Key design points vs v1:
 - Scores for the two heads run CONCURRENTLY in disjoint PE row-groups
   (K=64 each: rows 0-63 and 64-127), writing adjacent PSUM banks, so
   one exp activation covers both heads ([128, 1024] per k-tile).
 - All scale factors fold away: q tables carry w_q (8*0.125=1), k
   tables carry 8*w_k, and the per-token rsqrt factors are multiplied
   into qstore/kstore via K=1 broadcast matmuls, so exp has no scale AP.
 - RMS rsqrt = exp(-0.5*ln(sumsq+eps)) on ACT: ln+exp live in ONE
   activation table set -> zero table switches in the whole kernel.
   Sumsq lands at partitions {0,32,64,96} via mask matmuls so the K=1
   broadcast matmuls stay legal.
 - qkv/norm work for batch b+1 and projection for b-1 are interleaved
   into attention(b)'s PE-queue gaps (attention is ACT-bound).
 - Per-batch AllToAll (token stripes of 256) overlaps collectives with
   compute; each core owns stripe j of every batch for the projection.
"""
import os
import sys

import numpy as np
import ml_dtypes

for _p in ("/opt/trn_rl_repo", "/root/.axon_site/_ro/trn_rl_repo"):
    if os.path.isdir(_p) and _p not in sys.path:
        sys.path.append(_p)

import concourse.bass as bass
import concourse.mybir as mybir
from concourse import bacc, tile
from concourse.bass_utils import run_bass_kernel_spmd

BF16 = ml_dtypes.bfloat16
F32 = mybir.dt.float32
BF = mybir.dt.bfloat16
AF = mybir.ActivationFunctionType

NCORE = 8
B, N, C, H, HD = 4, 2048, 1024, 16, 64
HL = H // NCORE           # 2 heads per core
QKCH = HL * HD            # 128 q (or k) channels per core
T = B * N                 # 8192 tokens
ST = N // NCORE           # 256-token output stripe per core per batch
TOKC = 1024               # qkv/norm token chunk
QC = 512                  # attention q chunk
KT = N // 128             # k tiles per batch (16)
h2 = HD // 2
EPS_SUM = 64.0 * 1e-6     # eps on the 64-element sumsq

_BUILD_CACHE = {}


def _build():
    if "nc" in _BUILD_CACHE:
        return _BUILD_CACHE["nc"]
    nc = bacc.Bacc(None, target_bir_lowering=False, debug=True)

    xT_d = nc.declare_dram_parameter("xT", [128, 8, T], BF, isOutput=False)
    wqkv_d = nc.declare_dram_parameter("wqkv", [128, 8, 3 * QKCH], BF, isOutput=False)
    wp_d = nc.declare_dram_parameter("wp", [128, 8, C], BF, isOutput=False)
    bp_d = nc.declare_dram_parameter("bp", [1, C], BF, isOutput=False)
    cosq_d = nc.declare_dram_parameter("cosq", [128, N], BF, isOutput=False)
    sinq_d = nc.declare_dram_parameter("sinq", [128, N], BF, isOutput=False)
    cosk_d = nc.declare_dram_parameter("cosk", [128, N], BF, isOutput=False)
    sink_d = nc.declare_dram_parameter("sink", [128, N], BF, isOutput=False)
    eye_d = nc.declare_dram_parameter("eye", [128, 128], BF, isOutput=False)
    out_d = nc.declare_dram_parameter("out", [B, ST, C], F32, isOutput=True)

    # one AllToAll per (batch, 512-token q-chunk): stripe = 64-token
    # interleave so every chunk carries all 8 cores' slices
    a2a_in = [nc.dram_tensor(f"a2a_in{i}", [NCORE, QKCH, 64], BF)
              for i in range(4 * B)]
    a2a_out = [nc.dram_tensor(f"a2a_out{i}", [NCORE, QKCH, 64], BF)
               for i in range(4 * B)]

    with tile.TileContext(nc) as tc:
        with (
            tc.tile_pool(name="persist", bufs=1) as pp,
            tc.tile_pool(name="xt", bufs=4) as xtp,
            tc.tile_pool(name="nrm", bufs=2) as nrm,
            tc.tile_pool(name="rfp", bufs=4) as rfp,
            tc.tile_pool(name="att", bufs=3) as att,
            tc.tile_pool(name="shp", bufs=2) as shp,
            tc.tile_pool(name="gatp", bufs=2) as gatp,
            tc.tile_pool(name="drp", bufs=2) as drp,
            tc.tile_pool(name="scp", bufs=2, space="PSUM") as scp,
            tc.tile_pool(name="paccp", bufs=2, space="PSUM") as paccp,
            tc.tile_pool(name="psml", bufs=2, space="PSUM") as psml,
        ):
            # ---- resident tiles ----
            w_sb = pp.tile([128, 8, 3 * QKCH], BF)
            wp_sb = pp.tile([128, 8, C], BF)
            bp_sb = pp.tile([1, C], BF)
            rope_sb = pp.tile([128, 4, N], BF)          # cosq|sinq|cosk|sink
            qstore = pp.tile([QKCH, T], BF)
            kstore = pp.tile([QKCH, T], BF)
            # per head: [V (64) | ones (32)] so P@V emits the softmax
            # denominator pre-broadcast across 32 partitions
            VW = HD + 32
            vstore = pp.tile([128, T // 128, 2 * VW], BF)
            eye_sb = pp.tile([128, 128], BF)
            # block-diagonal ones: sumsq matvec output lands pre-broadcast
            # on all 128 partitions (rows 0-63 head0, 64-127 head1)
            maskD = pp.tile([128, 128], BF)
            ones_sb = pp.tile([128, HD], BF)            # K=1 lhsT rows at any partition
            ones1_128 = pp.tile([1, 128], BF)
            eps_col = pp.tile([128, 1], F32)            # rms eps as activation bias

            nc.sync.dma_start(w_sb[:], wqkv_d[:])

            def load_late_weights():
                # w_proj is first needed by proj(0) during attention(1);
                # keep its 2MB off the startup critical path
                nc.sync.dma_start(wp_sb[:], wp_d[:])
                nc.sync.dma_start(bp_sb[:], bp_d[:])

            def load_tables():
                nc.sync.dma_start(eye_sb[:], eye_d[:])
                for i, td in enumerate((cosq_d, sinq_d, cosk_d, sink_d)):
                    nc.sync.dma_start(rope_sb[:, i, :], td[:])
            nc.vector.memset(maskD[:], 0.0)
            nc.vector.memset(maskD[0:64, 0:64], 1.0)
            nc.vector.memset(maskD[64:128, 64:128], 1.0)
            nc.vector.memset(ones_sb[:], 1.0)
            nc.vector.memset(ones1_128[:], 1.0)
            nc.vector.memset(eps_col[:], EPS_SUM)
            nc.vector.memset(vstore[:, :, HD:VW], 1.0)
            nc.vector.memset(vstore[:, :, VW + HD:2 * VW], 1.0)

            # ---------- qkv + norm for one batch (two 1024-token chunks) ----
            # ACT-table discipline: all 4 Ln calls for the batch execute as
            # one consecutive cluster (one natural_log load), then the
            # exp(-0.5) calls rejoin the attention Exp stream (one exp load).
            # scheduler-order glue so Ln bursts stay contiguous between
            # attention Exps (minimizes ACT table-set reloads)
            sched = {"last": None, "bar": None}

            def _chain(instrs):
                for a, b2 in zip(instrs, instrs[1:]):
                    tile.add_dep_helper(
                        b2.ins, a.ins, sync=False, reason="act table cluster")

            def qkv_norm_items(b):
                items = []
                lns = []        # deferred matvec+ln closures, run as one item
                exps = []       # deferred exp(-0.5) closures
                tail = []       # rope + r-multiply items
                actins = []     # ACT instructions to keep contiguous
                for ti in (2 * b, 2 * b + 1):
                    tok0 = ti * TOKC
                    n0 = tok0 % N
                    st = {}

                    def xload(tok0=tok0, st=st):
                        for t5 in range(2):
                            tk0 = tok0 + t5 * 512
                            xt = xtp.tile([128, 8, 512], BF, tag="xt")
                            nc.sync.dma_start(xt[:], xT_d[:, :, tk0:tk0 + 512])
                            st[t5] = xt

                    def qk_group(t5, m, ti=ti, tok0=tok0, st=st):
                        tk0 = tok0 + t5 * 512
                        xt = st[t5]
                        store = qstore if m == 0 else kstore
                        ps = psml.tile([128, 512], F32, tag="psml")
                        for c in range(8):
                            nc.tensor.matmul(
                                ps[:], w_sb[:, c, m * QKCH:(m + 1) * QKCH],
                                xt[:, c, :], start=(c == 0), stop=(c == 7))
                        nc.vector.tensor_copy(store[:, tk0:tk0 + 512], ps[:])

                    def v_mm(t5, tok0=tok0, st=st):
                        # v as [vch, tok] (weights stationary), then cast to
                        # SBUF for the PE transposes
                        xt = st[t5]
                        ps = psml.tile([128, 512], F32, tag="psml")
                        for c in range(8):
                            nc.tensor.matmul(
                                ps[:], w_sb[:, c, 2 * QKCH:3 * QKCH],
                                xt[:, c, :], start=(c == 0), stop=(c == 7))
                        vtmp = nrm.tile([128, 512], BF, tag="vtmp")
                        nc.vector.tensor_copy(vtmp[:], ps[:])
                        st["v%d" % t5] = vtmp

                    def v_tr(t5, tok0=tok0, st=st):
                        tk0 = tok0 + t5 * 512
                        vtmp = st["v%d" % t5]
                        ps = psml.tile([128, 512], F32, tag="psml")
                        pt4 = ps[:].bitcast(BF)
                        for t1 in range(4):
                            nc.tensor.transpose(
                                pt4[:, t1 * 128:(t1 + 1) * 128],
                                vtmp[:, t1 * 128:(t1 + 1) * 128], eye_sb[:])
                        for t1 in range(4):
                            g = (tk0 // 128) + t1
                            nc.vector.tensor_copy(
                                vstore[:, g, :].rearrange(
                                    "p (a b) -> p a b", b=VW)[:, :, 0:HD],
                                pt4[:, t1 * 128:(t1 + 1) * 128].rearrange(
                                    "p (a b) -> p a b", b=HD))

                    def squares(tok0=tok0, st=st):
                        sqq = nrm.tile([128, TOKC], BF, tag="sqq")
                        sqk = nrm.tile([128, TOKC], BF, tag="sqk")
                        nc.vector.tensor_mul(
                            sqq[:], qstore[:, tok0:tok0 + TOKC], qstore[:, tok0:tok0 + TOKC])
                        nc.vector.tensor_mul(
                            sqk[:], kstore[:, tok0:tok0 + TOKC], kstore[:, tok0:tok0 + TOKC])
                        st["sqq"], st["sqk"] = sqq, sqk

                    def rf_ln(m, ch, st=st):
                        sq = st["sqq"] if m == 0 else st["sqk"]
                        ps = psml.tile([128, 512], F32, tag="psml")
                        nc.tensor.matmul(
                            ps[:], maskD[:], sq[:, ch * 512:(ch + 1) * 512],
                            start=True, stop=True)
                        lnscr = rfp.tile([128, 512], F32, tag="lnscr")
                        actins.append(nc.scalar.activation(
                            lnscr[:], ps[:], AF.Ln, bias=eps_col[:]))
                        st["ln%d%d" % (m, ch)] = lnscr

                    def rf_exp(m, ch, st=st):
                        rall = rfp.tile([128, 512], BF, tag="rall")
                        actins.append(nc.scalar.activation(
                            rall[:], st["ln%d%d" % (m, ch)], AF.Exp, scale=-0.5))
                        st["rall%d%d" % (m, ch)] = rall

                    def rope(m, tok0=tok0, n0=n0, st=st):
                        store = qstore if m == 0 else kstore
                        slf = store[:, tok0:tok0 + TOKC]
                        qrot = nrm.tile([128, TOKC], BF, tag="qrot")
                        for r0 in (0, HD):
                            nc.vector.tensor_copy(
                                qrot[r0:r0 + h2, :], store[r0 + h2:r0 + HD, tok0:tok0 + TOKC])
                            nc.vector.tensor_copy(
                                qrot[r0 + h2:r0 + HD, :], store[r0:r0 + h2, tok0:tok0 + TOKC])
                        cw = rope_sb[:, 2 * m, n0:n0 + TOKC]
                        sw = rope_sb[:, 2 * m + 1, n0:n0 + TOKC]
                        tms = nrm.tile([128, TOKC], BF, tag="tms")
                        nc.vector.tensor_mul(slf, slf, cw)
                        nc.vector.tensor_mul(tms[:], qrot[:], sw)
                        nc.vector.tensor_add(slf, slf, tms[:])

                    def rmul(m, tok0=tok0, st=st):
                        # multiply the pre-broadcast rsqrt into the store
                        store = qstore if m == 0 else kstore
                        for ch in range(2):
                            sl = store[:, tok0 + ch * 512:tok0 + (ch + 1) * 512]
                            nc.vector.tensor_mul(sl, sl, st["rall%d%d" % (m, ch)][:])

                    items.append((xload, squares, qk_group, v_mm, v_tr))
                    lns.append(lambda f=rf_ln: (f(0, 0), f(0, 1), f(1, 0), f(1, 1)))
                    exps.append(lambda f=rf_exp: (f(0, 0), f(0, 1), f(1, 0), f(1, 1)))
                    tail += [
                        lambda f=rope: f(0),
                        lambda f=rmul: f(0),
                        lambda f=rope: f(1),
                        lambda f=rmul: f(1),
                    ]
                def mk_cluster(run):
                    def item():
                        start = len(actins)
                        run()
                        seq = actins[start:]
                        head = sched["last"] or sched["bar"]
                        if head is not None:
                            tile.add_dep_helper(
                                seq[0].ins, head.ins,
                                sync=False, reason="act cluster head")
                        _chain(seq)
                        sched["bar"] = seq[-1]
                    return item

                (xl0, sq0, qk0, vm0, vt0), (xl1, sq1, qk1, vm1, vt1) = items
                # (slot, closure): DMA-only prefetch first so MM groups never
                # head-of-line block the PE queue on an HBM load; one short
                # PE burst per slot so the exp stream never starves
                slotted = [
                    (0, xl0), (1, xl1),
                    (2, lambda: qk0(0, 0)), (3, lambda: qk0(0, 1)),
                    (4, lambda: qk0(1, 0)), (5, lambda: qk0(1, 1)),
                    (6, lambda: qk1(0, 0)), (7, lambda: qk1(0, 1)),
                    (8, lambda: qk1(1, 0)), (9, lambda: qk1(1, 1)),
                    (10, sq0),
                    (11, lambda: vm0(0)), (12, lambda: vt0(0)),
                    (13, lambda: vm0(1)), (14, lambda: vt0(1)),
                    (15, sq1),
                    (16, lambda: vm1(0)), (17, lambda: vt1(0)),
                    (18, lambda: vm1(1)), (19, lambda: vt1(1)),
                    (20, mk_cluster(lns[0])),
                    (22, mk_cluster(exps[0])),
                    (24, mk_cluster(lns[1])),
                    (26, mk_cluster(exps[1])),
                ]
                slotted += [(28 + 2 * i, f) for i, f in enumerate(tail)]
                return slotted

            # ---------- attention for one batch ----------
            def attention(b, feeder):
                boff = b * N
                shard = shp.tile([QKCH, N], BF, tag="shard")
                fi = 0
                slot = 0
                prev = [None]

                def drain1(pr):
                    p0, p1, qc = pr
                    den0 = drp.tile([32, QC], F32, tag="den0")
                    den1 = drp.tile([32, QC], F32, tag="den1")
                    nc.vector.tensor_copy(den0[:], p0[64:96, :])
                    nc.vector.tensor_copy(den1[:], p1[64:96, :])
                    pvs = drp.tile([128, QC], BF, tag="pvs")
                    nc.vector.tensor_copy(pvs[0:64, :], p0[0:64, :])
                    nc.vector.tensor_copy(pvs[64:128, :], p1[0:64, :])
                    pr += [den0, den1, pvs]

                def drain2(pr):
                    den0, den1 = pr[3], pr[4]
                    drec0 = drp.tile([32, QC], F32, tag="drec0")
                    drec1 = drp.tile([32, QC], F32, tag="drec1")
                    with nc.allow_low_precision(reason="softmax denom"):
                        nc.vector.reciprocal_approx_fast(drec0[:], den0[:])
                        nc.vector.reciprocal_approx_fast(drec1[:], den1[:])
                    d01 = drp.tile([128, QC], BF, tag="d01")
                    nc.vector.tensor_copy(d01[0:32, :], drec0[:])
                    nc.vector.tensor_copy(d01[32:64, :], drec0[:])
                    nc.vector.tensor_copy(d01[64:96, :], drec1[:])
                    nc.vector.tensor_copy(d01[96:128, :], drec1[:])
                    pr.append(d01)

                def drain3(pr):
                    _p0, _p1, qc, _d0f, _d1f, pvs, d01 = pr
                    nc.vector.tensor_mul(
                        shard[:, qc * QC:(qc + 1) * QC], pvs[:], d01[:])
                    # stage + AllToAll this q-chunk immediately (64-token
                    # stripes: chunk j goes to core j); keeps the collective
                    # entirely off the critical tail
                    idx = 4 * b + qc
                    nc.gpsimd.dma_start(
                        a2a_in[idx][:].rearrange("j p t -> p j t"),
                        shard[:, qc * QC:(qc + 1) * QC].rearrange(
                            "p (j t) -> p j t", j=NCORE))
                    nc.gpsimd.collective_compute(
                        "AllToAll",
                        mybir.AluOpType.bypass,
                        replica_groups=[list(range(NCORE))],
                        ins=[a2a_in[idx][:]],
                        outs=[a2a_out[idx][:]],
                    )

                for qc in range(N // QC):
                    qoff = boff + qc * QC
                    p0 = paccp.tile([96, QC], F32, tag="pacc")
                    p1 = paccp.tile([96, QC], F32, tag="pacc")
                    pend = []      # pts awaiting PV, two k-tiles behind exp

                    def pv_flush(last):
                        while pend and (len(pend) > 2 or last):
                            gp, ptp = pend.pop(0)
                            nc.tensor.matmul(
                                p0[:], vstore[:, gp, 0:VW], ptp[:, 0, :],
                                start=(gp % KT == 0), stop=(last and not pend))
                            nc.tensor.matmul(
                                p1[:], vstore[:, gp, VW:2 * VW], ptp[:, 1, :],
                                start=(gp % KT == 0), stop=(last and not pend))

                    for kt in range(KT):
                        koff = boff + kt * 128
                        g = koff // 128
                        # feeder work goes ahead of the score MMs: it fills
                        # the PE-queue wait on the score-buffer WAR
                        if fi < len(feeder) and feeder[fi][0] <= slot:
                            feeder[fi][1]()
                            fi += 1
                        sc = scp.tile([128, 2, QC], F32, tag="sc")
                        nc.tensor.matmul(
                            sc[:, 0, :], kstore[0:HD, koff:koff + 128],
                            qstore[0:HD, qoff:qoff + QC], start=True, stop=True)
                        nc.tensor.matmul(
                            sc[:, 1, :], kstore[HD:128, koff:koff + 128],
                            qstore[HD:128, qoff:qoff + QC], start=True, stop=True)
                        if prev[0] is not None:
                            if kt == 0:
                                drain1(prev[0])
                            elif kt == 1:
                                drain2(prev[0])
                            elif kt == 2:
                                drain3(prev[0])
                                prev[0] = None
                        pt = att.tile([128, 2, QC], BF, tag="pt")
                        e = nc.scalar.activation(pt[:], sc[:], AF.Exp)
                        if sched["bar"] is not None:
                            tile.add_dep_helper(
                                e.ins, sched["bar"].ins,
                                sync=False, reason="act cluster barrier")
                            sched["bar"] = None
                        sched["last"] = e
                        pend.append((g, pt))
                        pv_flush(False)
                        slot += 1
                    pv_flush(True)
                    prev[0] = [p0, p1, qc]
                # flush remaining feeder + final drain
                while fi < len(feeder):
                    feeder[fi][1]()
                    fi += 1
                drain1(prev[0])
                drain2(prev[0])
                drain3(prev[0])
                return shard

            def proj_items(b):
                st = {}

                def gather(qc):
                    if "g" not in st:
                        gat = gatp.tile([128, 8, ST], BF, tag="gat")
                        st["g"] = gat
                    nc.gpsimd.dma_start(
                        st["g"][:, :, qc * 64:(qc + 1) * 64],
                        a2a_out[4 * b + qc][:].rearrange("c p t -> p c t"))

                def mmgroup(tg, d5):
                    gat = st["g"]
                    ps = psml.tile([128, 512], F32, tag="psml")
                    for c in range(8):
                        nc.tensor.matmul(
                            ps[:], gat[:, c, tg * 128:(tg + 1) * 128],
                            wp_sb[:, c, d5 * 512:(d5 + 1) * 512],
                            start=(c == 0), stop=(c == 7))
                    ysb = gatp.tile([128, 512], F32, tag="ysb")
                    nc.vector.tensor_copy(ysb[:], ps[:])
                    nc.gpsimd.dma_start(
                        out_d[b, tg * 128:(tg + 1) * 128, d5 * 512:(d5 + 1) * 512],
                        ysb[:])

                return [(lambda qc=qc: gather(qc)) for qc in range(4)] + [
                    (lambda tg=tg, d5=d5: mmgroup(tg, d5))
                    for tg in range(2) for d5 in range(2)]

            # ---------- main pipeline ----------
            items0 = sorted(qkv_norm_items(0), key=lambda x: x[0])
            items0[0][1]()          # x prefetch right behind the qkv weights
            items0[1][1]()
            load_tables()
            for _s, it in items0[2:]:
                it()
            for b in range(B):
                feeder = []
                if b + 1 < B:
                    feeder += qkv_norm_items(b + 1)
                if b == 0:
                    feeder.append((27, load_late_weights))
                if b >= 1:
                    pj = proj_items(b - 1)
                    feeder += list(zip((2, 4, 6, 10, 16, 20, 24, 28), pj))
                feeder.sort(key=lambda x: x[0])
                attention(b, feeder)
            for f in proj_items(B - 1):
                f()

    nc.compile()
    _BUILD_CACHE["nc"] = nc
    return nc


def _host_prep(x, rope_cos, rope_sin, w_qkv, w_proj, b_proj, q_norm_w, k_norm_w):
    x = np.asarray(x, np.float32)
    xT = np.ascontiguousarray(
        x.reshape(T, C).T.reshape(8, 128, T).transpose(1, 0, 2)).astype(BF16)
    cosT = np.asarray(rope_cos, np.float32)[0, 0].T          # [hd, N]
    sinT = np.asarray(rope_sin, np.float32)[0, 0].T

    def fold(w, s):
        w = np.asarray(w, np.float32)
        cw = cosT * w[:, None] * s
        sw = np.empty_like(sinT)
        sw[:h2] = -sinT[:h2] * w[h2:HD, None] * s
        sw[h2:] = sinT[h2:] * w[0:h2, None] * s
        dup = lambda a: np.ascontiguousarray(np.concatenate([a, a], 0)).astype(BF16)
        return dup(cw), dup(sw)

    cosq, sinq = fold(q_norm_w, 1.0)     # 8 (rms) * 0.125 (softmax scale) = 1
    cosk, sink = fold(k_norm_w, 8.0)
    w_proj = np.asarray(w_proj, np.float32)
    wp = np.ascontiguousarray(
        w_proj.T.reshape(8, 128, C).transpose(1, 0, 2)).astype(BF16)
    bp = np.asarray(b_proj, np.float32).reshape(1, C).astype(BF16)
    w_qkv = np.asarray(w_qkv, np.float32)

    in_maps = []
    for r in range(NCORE):
        wq = w_qkv[QKCH * r:QKCH * (r + 1), :].T
        wk = w_qkv[C + QKCH * r:C + QKCH * (r + 1), :].T
        wv = w_qkv[2 * C + QKCH * r:2 * C + QKCH * (r + 1), :].T
        wqkvT = np.concatenate([wq, wk, wv], axis=1)         # [C, 384]
        wqkv = np.ascontiguousarray(
            wqkvT.reshape(8, 128, 3 * QKCH).transpose(1, 0, 2)).astype(BF16)
        in_maps.append({
            "xT": xT, "wqkv": wqkv, "wp": wp, "bp": bp,
            "cosq": cosq, "sinq": sinq, "cosk": cosk, "sink": sink,
            "eye": np.eye(128, dtype=BF16),
        })
    return in_maps


def _run(in_maps, trace=False, **kwargs):
    nc = _build()
    return run_bass_kernel_spmd(
        nc, in_maps, core_ids=list(range(NCORE)), trace=trace, **kwargs)


def _unshard(res):
    # core j's rows are (qc, 64) for 64-token interleaved stripes:
    # y[b, qc*512 + j*64 + t] = out_j[b, qc*64 + t]
    y = np.empty((B, 4, NCORE, 64, C), np.float32)
    for j in range(NCORE):
        y[:, :, j] = np.asarray(
            res.results[j]["out"], np.float32).reshape(B, 4, 64, C)
    return np.ascontiguousarray(y.reshape(B, N, C))


def kernel(**inputs):
    in_maps = _host_prep(**inputs)
    res = _run(in_maps)
    y = _unshard(res)
    # projection bias applied on host (zeros by spec, but honor it anyway)
    return y + np.asarray(inputs["b_proj"], np.float32)[None, None, :]
